# revision 15
# baseline (speedup 1.0000x reference)
"""Trainium2 Bass kernel for 2-layer GATv2 (N=50000, E=800000, 128->64->64->2).

Strategy (edge-parallel, dst-sharded, 8 NeuronCores):
  * Host sorts edges by dst; core c owns dst nodes [c*N/8, (c+1)*N/8).
  * The softmax denominator factors out of the weighted sum, so each layer is
    ONE edge pass: gather fs[src], fd[dst]; score = a . lrelu(fs+fd);
    e = exp(score) (max-subtraction skipped -- scores are O(1)); a 0/1
    selection-matrix matmul scatter-adds [e*fs[src] | e] into per-128-node
    window PSUM accumulators; h = relu(u/s).
  * fs tables are per-core-replicated (src is global); fd tables are local.
  * dma_gather (Q7 SWDGE, int16 idx): fs indices split lo/hi at 32768; edges
    within each window group are reordered lo-first (sums are order-invariant).
  * Between layers: AllGather of h1^T pieces (ncfw collective).
"""
import sys
import numpy as np

sys.path.insert(0, "/opt/trn_rl_repo")

import ml_dtypes

BF16 = ml_dtypes.bfloat16

# ---------------- problem constants (hardcoded per contract) ----------------
N = 50000
E = 800000
IN_F = 128
HF = 64          # hidden feats
HEADS = 4
DH = 16
NEG_SLOPE = 0.2
N_CORES = 8
NB = N // N_CORES            # nodes per core
WIN = 128                    # window size (nodes)
WPC = (NB + WIN - 1) // WIN  # windows per core
GRP = 4                      # windows per psum group
ST = 16                      # tiles per DVE supertile
LO_SPLIT = 25000             # fs index split (balances q0/q1 descriptor load)
P = 128

_CACHE = {}
MAX_PHASE = 6
JUNK_SAFE = False
USE_PRELU = True  # cayman exp_and_others table holds Exp+Prelu+Copy+Relu together
EDGE_LEVEL = 3
REPEAT = 1
MAXJ = 32  # max scatter jobs per supertile chunk (batched sel build width)


def _wrap16(vals):
    """int array [n] (n % 16 == 0) -> [128, n/16] int16 wrapped+replicated."""
    b = vals.reshape(-1, 16).T.astype(np.int16)
    return np.tile(b, (8, 1))


def _prep_edges(src, dst):
    """Sort by dst, shard by dst range, group-level lo/hi packing.

    Stream per group of GRP windows: [lo(w0)..lo(w3) | hi(w0)..hi(w3)], padded
    to 128-edge tiles only at the lo/hi block level. A tile may span several
    windows; the per-(tile,window) scatter matmuls are emitted as "jobs" with
    their own dst-rel column (-1 outside the window).
    """
    src = np.asarray(src, dtype=np.int64)
    dst = np.asarray(dst, dtype=np.int64)
    perm = np.argsort(dst, kind="stable")
    se, de = src[perm], dst[perm]
    per_cw = [[None] * WPC for _ in range(N_CORES)]
    for c in range(N_CORES):
        a = np.searchsorted(de, c * NB, side="left")
        b = np.searchsorted(de, (c + 1) * NB, side="left")
        s_c, r_c = se[a:b], de[a:b] - c * NB
        w_c = r_c // WIN
        for w in range(WPC):
            m = w_c == w
            s_w, r_w = s_c[m], r_c[m]
            lo = s_w < LO_SPLIT
            per_cw[c][w] = (s_w[lo], r_w[lo], s_w[~lo], r_w[~lo])

    groups = []
    for g0 in range(0, WPC, GRP):
        ws = list(range(g0, min(g0 + GRP, WPC)))
        # per-core per-seg edge counts -> group tile counts (max over cores)
        lo_tot = [sum(len(per_cw[c][w][0]) for w in ws) for c in range(N_CORES)]
        hi_tot = [sum(len(per_cw[c][w][2]) for w in ws) for c in range(N_CORES)]
        T_lo = max(-(-n // P) for n in lo_tot)
        T_hi = max(-(-n // P) for n in hi_tot)
        gt = T_lo + T_hi
        # jobs: union over cores of (tile, w) touched
        jobs_set = {}
        for c in range(N_CORES):
            pos = 0
            for seg, base in ((0, 0), (2, T_lo * P)):
                pos = base
                for w in ws:
                    n = len(per_cw[c][w][seg])
                    if n:
                        for t in range(pos // P, -(-(pos + n) // P)):
                            jobs_set[(t, w)] = True
                    pos += n
        jobs = sorted(jobs_set.keys())
        first_j, last_j = {}, {}
        for j, (t, w) in enumerate(jobs):
            if w not in first_j:
                first_j[w] = j
            last_j[w] = j
        groups.append({"ws": ws, "gt": gt, "T_lo": T_lo, "T_hi": T_hi,
                       "jobs": jobs, "first_j": first_j, "last_j": last_j})
    TT = sum(g["gt"] for g in groups)
    NJ = sum(len(g["jobs"]) for g in groups)

    fs_idx = np.zeros((N_CORES, P, TT * 8), np.int16)
    fd_idx = np.zeros((N_CORES, P, TT * 8), np.int16)
    dstw = np.full((N_CORES, P, NJ), -1.0, np.float32)
    for c in range(N_CORES):
        col = 0
        j_base = 0
        for g in groups:
            gt, T_lo = g["gt"], g["T_lo"]
            s_all = np.zeros(gt * P, np.int64)
            d_all = np.zeros(gt * P, np.int64)
            wof = np.full(gt * P, -1, np.int64)   # window of each slot
            r_all = np.zeros(gt * P, np.int64)
            for seg, base in ((0, 0), (2, T_lo * P)):
                pos = base
                for w in g["ws"]:
                    s_w = per_cw[c][w][seg]
                    r_w = per_cw[c][w][seg + 1]
                    n = len(s_w)
                    s_all[pos:pos + n] = s_w - (LO_SPLIT if seg else 0)
                    d_all[pos:pos + n] = r_w
                    r_all[pos:pos + n] = r_w
                    wof[pos:pos + n] = w
                    pos += n
            fs_idx[c, :, col:col + T_lo * 8] = _wrap16(s_all[:T_lo * P])
            if gt - T_lo:
                fs_idx[c, :, col + T_lo * 8:col + gt * 8] = \
                    _wrap16(s_all[T_lo * P:])
            fd_idx[c, :, col:col + gt * 8] = _wrap16(d_all)
            col += gt * 8
            # dstw per job
            for j, (t, w) in enumerate(g["jobs"]):
                sl = slice(t * P, (t + 1) * P)
                v = np.where(wof[sl] == w, r_all[sl] - w * WIN, -1.0)
                dstw[c, :, j_base + j] = v
            j_base += len(g["jobs"])
    return {"groups": groups, "TT": TT, "NJ": NJ}, fs_idx, fd_idx, dstw


def _build_program(sched):
    import concourse.bacc as bacc
    import concourse.mybir as mybir
    import concourse.tile as tile

    BF = mybir.dt.bfloat16
    F32 = mybir.dt.float32
    I16 = mybir.dt.int16
    AF = mybir.ActivationFunctionType
    OP = mybir.AluOpType
    AX = mybir.AxisListType

    TT = sched["TT"]
    NJ = sched["NJ"]
    groups = sched["groups"]

    nc = bacc.Bacc("TRN2", target_bir_lowering=False, debug=False,
                   num_devices=N_CORES, num_swdge_queues=4)

    featT = nc.dram_tensor("featT", [IN_F, N], BF, kind="ExternalInput").ap()
    featT_own = nc.dram_tensor("featT_own", [IN_F, NB], BF,
                               kind="ExternalInput").ap()
    fs_idx_d = nc.dram_tensor("fs_idx", [P, TT * 8], I16,
                              kind="ExternalInput").ap()
    fd_idx_d = nc.dram_tensor("fd_idx", [P, TT * 8], I16,
                              kind="ExternalInput").ap()
    dstw_d = nc.dram_tensor("dstw", [P, NJ], BF, kind="ExternalInput").ap()
    wfs1_d = nc.dram_tensor("wfs1", [IN_F, HF], BF, kind="ExternalInput").ap()
    wfd1_d = nc.dram_tensor("wfd1", [IN_F, HF], BF, kind="ExternalInput").ap()
    ws2_d = nc.dram_tensor("ws2", [HF, HF], BF, kind="ExternalInput").ap()
    wd2_d = nc.dram_tensor("wd2", [HF, HF], BF, kind="ExternalInput").ap()
    bias_d = nc.dram_tensor("bias", [P, 4, HF], BF, kind="ExternalInput").ap()
    arep_d = nc.dram_tensor("arep", [P, 2, HF], BF, kind="ExternalInput").ap()
    iota_rep_d = nc.dram_tensor("iota_rep", [P, MAXJ * P], BF,
                                kind="ExternalInput").ap()
    ident_d = nc.dram_tensor("ident", [P, P], BF, kind="ExternalInput").ap()
    wout_d = nc.dram_tensor("wout", [HF, 2], BF, kind="ExternalInput").ap()
    bout_d = nc.dram_tensor("bout", [2, 1], F32, kind="ExternalInput").ap()
    outT_d = nc.dram_tensor("outT", [2, NB], F32, kind="ExternalOutput").ap()

    fs1_t = nc.dram_tensor("fs1_t", [N, P], BF).ap()   # cols 0:64 live
    fd1_t = nc.dram_tensor("fd1_t", [NB, P], BF).ap()
    fs2_own = nc.dram_tensor("fs2_own", [NB, P], BF).ap()
    fs2_t = nc.dram_tensor("fs2_t", [N, P], BF, addr_space="Shared").ap()
    fd2_t = nc.dram_tensor("fd2_t", [NB, P], BF).ap()

    with tile.TileContext(nc) as tc:
        with (
            tc.tile_pool(name="const", bufs=1) as cpool,
            tc.tile_pool(name="work", bufs=2) as wpool,
            tc.tile_pool(name="gath", bufs=2) as gpool,
        ):
            def cload(name, shape, dt_, src_ap):
                t = cpool.tile(shape, dt_, tag=name)
                nc.sync.dma_start(out=t[:], in_=src_ap)
                return t

            dstw_sb = cload("dstw_sb", [P, NJ], BF, dstw_d[:, :])
            wfs1_sb = cload("wfs1_sb", [IN_F, HF], BF, wfs1_d[:, :])
            wfd1_sb = cload("wfd1_sb", [IN_F, HF], BF, wfd1_d[:, :])
            ws2_sb = cload("ws2_sb", [HF, HF], BF, ws2_d[:, :])
            wd2_sb = cload("wd2_sb", [HF, HF], BF, wd2_d[:, :])
            bias_sb = cload("bias_sb", [P, 4, HF], BF, bias_d[:, :, :])
            arep_sb = cload("arep_sb", [P, 2, HF], BF, arep_d[:, :, :])
            iota_rep_sb = cload("iota_rep_sb", [P, MAXJ * P], BF,
                                iota_rep_d[:, :])
            ident_sb = cload("ident_sb", [P, P], BF, ident_d[:, :])
            wout_sb = cload("wout_sb", [HF, 2], BF, wout_d[:, :])
            bout_sb = cload("bout_sb", [2, 1], F32, bout_d[:, :])
            h1T_own = cpool.tile([HF, NB], BF, tag="h1T_own")
            h2T_own = cpool.tile([HF, NB], BF, tag="h2T_own")

            def project(psp, dst_table, n_rows, row0, lhsT_of, w_sb, bias_idx):
                """dst_table[row0+i, 0:64] = lhsT(i)^T @ w + bias (batches)."""
                BATCH = 8 * P
                for b0 in range(0, n_rows, BATCH):
                    bn = min(BATCH, n_rows - b0)
                    nch = -(-bn // P)
                    ps = psp.tile([P, 8 * HF], F32, tag="proj_psum",
                                  space="PSUM")
                    for k in range(nch):
                        c0 = b0 + k * P
                        cn = min(P, n_rows - c0)
                        nc.tensor.matmul(
                            out=ps[0:cn, k * HF:(k + 1) * HF],
                            lhsT=lhsT_of(c0, cn), rhs=w_sb[:],
                            start=True, stop=True)
                    ob = wpool.tile([P, 8, P], BF, tag="proj_out")
                    if JUNK_SAFE:
                        nc.vector.memset(ob[:, :, HF:P], 0.0)
                    wcols = P if JUNK_SAFE else HF
                    nc.vector.tensor_add(
                        out=ob[:, 0:nch, 0:HF],
                        in0=ps[:].rearrange("p (k f) -> p k f", k=8)[:, 0:nch, :],
                        in1=bias_sb[:, bias_idx, :].unsqueeze(1)
                            .to_broadcast([P, nch, HF]))
                    nf = bn // P
                    if nf:
                        nc.sync.dma_start(
                            out=dst_table[row0 + b0:row0 + b0 + nf * P, 0:wcols]
                                .rearrange("(k p) f -> p k f", p=P),
                            in_=ob[:, 0:nf, 0:wcols])
                    if bn - nf * P:
                        nc.sync.dma_start(
                            out=dst_table[row0 + b0 + nf * P:row0 + b0 + bn,
                                          0:wcols],
                            in_=ob[0:bn - nf * P, nf, 0:wcols])

            def edge_layer(win_ps, hT_ps_pool, fs_table, fd_table, a_idx,
                           hT_own):
                t_base = 0
                col = 0
                j_base = 0
                for g in groups:
                    gt = g["gt"]
                    n_lo = g["T_lo"]
                    fsg = gpool.tile([P, gt, P], BF, tag="fsg")
                    fdg = gpool.tile([P, gt, P], BF, tag="fdg")
                    fs_ix = gpool.tile([P, gt * 8], I16, tag="fs_ix")
                    nc.sync.dma_start(out=fs_ix[:],
                                      in_=fs_idx_d[:, col:col + gt * 8])
                    fd_ix = gpool.tile([P, gt * 8], I16, tag="fd_ix")
                    nc.sync.dma_start(out=fd_ix[:],
                                      in_=fd_idx_d[:, col:col + gt * 8])
                    gt2 = gt // 2
                    if gt2 and EDGE_LEVEL >= 0:
                        nc.gpsimd.dma_gather(
                            fdg[:, 0:gt2, :], fd_table[:, :],
                            fd_ix[:, 0:gt2 * 8], gt2 * P, gt2 * P, P,
                            single_packet=False, queue_num=2)
                    if gt - gt2 and EDGE_LEVEL >= 0:
                        nc.gpsimd.dma_gather(
                            fdg[:, gt2:gt, :], fd_table[:, :],
                            fd_ix[:, gt2 * 8:gt * 8],
                            (gt - gt2) * P, (gt - gt2) * P, P,
                            single_packet=False, queue_num=3)
                    if n_lo and EDGE_LEVEL >= 0:
                        nc.gpsimd.dma_gather(
                            fsg[:, 0:n_lo, :], fs_table[0:LO_SPLIT, :],
                            fs_ix[:, 0:n_lo * 8], n_lo * P, n_lo * P, P,
                            single_packet=False, queue_num=0)
                    if gt - n_lo and EDGE_LEVEL >= 0:
                        nc.gpsimd.dma_gather(
                            fsg[:, n_lo:gt, :], fs_table[LO_SPLIT:N, :],
                            fs_ix[:, n_lo * 8:gt * 8],
                            (gt - n_lo) * P, (gt - n_lo) * P, P,
                            single_packet=False, queue_num=1)
                    col += gt * 8

                    jobs = g["jobs"]
                    first_j, last_j = g["first_j"], g["last_j"]
                    psums = {w: win_ps.tile([P, HF + HEADS], F32, name="win_psum",
                                            tag="win_psum", space="PSUM")
                             for w in first_j}

                    for s0 in range(0, gt, ST):
                        if EDGE_LEVEL < 1:
                            break
                        sn = min(ST, gt - s0)
                        fs_v = fsg[:, s0:s0 + sn, 0:HF]
                        fd_v = fdg[:, s0:s0 + sn, 0:HF]
                        t0 = wpool.tile([P, ST, HF], BF, tag="t0")
                        nc.vector.tensor_add(out=t0[:, 0:sn, :], in0=fs_v,
                                             in1=fd_v)
                        t1 = wpool.tile([P, ST, HF], BF, tag="t1")
                        if USE_PRELU:
                            nc.scalar.activation(
                                out=t1[:, 0:sn, :], in_=t0[:, 0:sn, :],
                                func=AF.Prelu, alpha=NEG_SLOPE)
                        else:
                            nc.vector.scalar_tensor_tensor(
                                out=t1[:, 0:sn, :], in0=t0[:, 0:sn, :],
                                scalar=NEG_SLOPE, in1=t0[:, 0:sn, :],
                                op0=OP.mult, op1=OP.max)
                        t2 = wpool.tile([P, ST, HF], BF, tag="t2")
                        nc.vector.tensor_mul(
                            out=t2[:, 0:sn, :], in0=t1[:, 0:sn, :],
                            in1=arep_sb[:, a_idx, :].unsqueeze(1)
                                .to_broadcast([P, sn, HF]))
                        t3 = wpool.tile([P, ST, HEADS, DH // 2], BF,
                                        tag="t3")
                        t2v = t2[:, 0:sn, :].rearrange(
                            "p t (h d) -> p (t h) d", d=DH)
                        nc.vector.tensor_add(
                            out=t3[:, 0:sn, :, :]
                                .rearrange("p t h d -> p (t h) d"),
                            in0=t2v[:, :, 0:DH // 2],
                            in1=t2v[:, :, DH // 2:DH])
                        sc = wpool.tile([P, ST * HEADS], F32, tag="sc")
                        nc.vector.tensor_reduce(
                            out=sc[:, 0:sn * HEADS]
                                .rearrange("p (t h) -> p t h", h=HEADS),
                            in_=t3[:, 0:sn, :, :]
                                .rearrange("p t h d -> p (t h) d"),
                            op=OP.add, axis=AX.X)
                        rhs = wpool.tile([P, ST, HF + HEADS], BF, tag="rhs")
                        nc.scalar.activation(
                            out=rhs[:, 0:sn, HF:HF + HEADS],
                            in_=sc[:, 0:sn * HEADS]
                                .rearrange("p (t h) -> p t h", h=HEADS),
                            func=AF.Exp)
                        nc.vector.tensor_mul(
                            out=rhs[:, 0:sn, 0:HF]
                                .rearrange("p t (h d) -> p t h d", d=DH),
                            in0=fs_v.rearrange("p t (h d) -> p t h d", d=DH),
                            in1=rhs[:, 0:sn, HF:HF + HEADS].unsqueeze(3)
                                .to_broadcast([P, sn, HEADS, DH]))
                        chunk_jobs = [(j, t, w) for j, (t, w) in
                                      enumerate(jobs) if s0 <= t < s0 + sn]
                        nJ = len(chunk_jobs)
                        assert nJ <= MAXJ, (nJ, MAXJ)
                        sel = wpool.tile([P, MAXJ, P], BF, tag="sel")
                        if EDGE_LEVEL >= 2 and nJ:
                            j0c = chunk_jobs[0][0]
                            nc.vector.tensor_tensor(
                                out=sel[:, 0:nJ, :],
                                in0=iota_rep_sb[:, 0:nJ * P]
                                    .rearrange("p (j n) -> p j n", n=P),
                                in1=dstw_sb[:, j_base + j0c:
                                            j_base + j0c + nJ]
                                    .unsqueeze(2).to_broadcast([P, nJ, P]),
                                op=OP.is_equal)
                            for js, (j, t, w) in enumerate(chunk_jobs):
                                nc.tensor.matmul(
                                    out=psums[w][:], lhsT=sel[:, js, :],
                                    rhs=rhs[:, t - s0, :],
                                    start=(j == first_j[w]),
                                    stop=(j == last_j[w]))

                    for w in g["ws"]:
                        if w not in first_j or EDGE_LEVEL < 2:
                            continue
                        ps = psums[w]
                        nw = min(WIN, NB - w * WIN)
                        s_eps = wpool.tile([P, HEADS], F32, tag="s_eps")
                        nc.vector.tensor_scalar_add(
                            out=s_eps[:], in0=ps[:, HF:HF + HEADS],
                            scalar1=1e-20)
                        s_inv = wpool.tile([P, HEADS], F32, tag="s_inv")
                        nc.vector.reciprocal(out=s_inv[:], in_=s_eps[:])
                        hw_ = wpool.tile([P, HF], BF, tag="hw_")
                        nc.vector.tensor_mul(
                            out=hw_[:].rearrange("p (h d) -> p h d", d=DH),
                            in0=ps[:, 0:HF].rearrange("p (h d) -> p h d",
                                                      d=DH),
                            in1=s_inv[:].unsqueeze(2)
                                .to_broadcast([P, HEADS, DH]))
                        hrel = wpool.tile([P, HF], BF, tag="hrel")
                        nc.scalar.activation(out=hrel[:], in_=hw_[:],
                                             func=AF.Relu)
                        if EDGE_LEVEL < 3:
                            continue
                        hT_ps = hT_ps_pool.tile([HF, P], BF, tag="hT_ps",
                                                space="PSUM")
                        nc.tensor.transpose(out=hT_ps[:], in_=hrel[:],
                                            identity=ident_sb[:])
                        nc.vector.tensor_copy(
                            out=hT_own[:, w * WIN:w * WIN + nw],
                            in_=hT_ps[:, 0:nw])
                    t_base += gt
                    j_base += len(jobs)

            def batched_lhsT(src_ap, width, tag):
                cache = {}

                def f(c0, cn):
                    b0 = (c0 // (8 * P)) * (8 * P)
                    if cache.get("b0") != b0:
                        bw = min(8 * P, width - b0)
                        t = wpool.tile([src_ap.shape[0], 8 * P], BF, tag=tag)
                        nc.sync.dma_start(out=t[:, 0:bw],
                                          in_=src_ap[:, b0:b0 + bw])
                        cache["b0"], cache["t"] = b0, t
                    return cache["t"][:, c0 - b0:c0 - b0 + cn]
                return f

            # ---- phase 1: layer-1 projections ----
            max_phase = MAX_PHASE
            for _rep in range(REPEAT):
              with tc.tile_pool(name="ps1", bufs=2, space="PSUM") as psp:
                  project(psp, fd1_t, NB, 0,
                          batched_lhsT(featT_own, NB, "featT_own_chunk"),
                          wfd1_sb, 1)
                  project(psp, fs1_t, N, 0,
                          batched_lhsT(featT, N, "featT_chunk"), wfs1_sb, 0)

              # ---- phase 2: layer-1 edge pass ----
              if max_phase >= 2:
                with (tc.tile_pool(name="wps1", bufs=6, space="PSUM") as win_ps,
                    tc.tile_pool(name="tps1", bufs=2, space="PSUM") as t_ps):
                  edge_layer(win_ps, t_ps, fs1_t, fd1_t, 0, h1T_own)

              # ---- phase 3+4: layer-2 projections (own rows) + AllGather ----
              if max_phase >= 3:
                with tc.tile_pool(name="ps2", bufs=2, space="PSUM") as psp:
                  project(psp, fd2_t, NB, 0,
                          lambda c0, cn: h1T_own[:, c0:c0 + cn], wd2_sb, 3)
                  project(psp, fs2_own, NB, 0,
                          lambda c0, cn: h1T_own[:, c0:c0 + cn], ws2_sb, 2)
                nc.gpsimd.collective_compute(
                  "AllGather", OP.bypass, ins=[fs2_own[:, :]],
                  outs=[fs2_t[:, :]],
                  replica_groups=[list(range(N_CORES))])

              # ---- phase 5: layer-2 edge pass ----
              if max_phase >= 4:
                with (tc.tile_pool(name="wps2", bufs=6, space="PSUM") as win_ps,
                    tc.tile_pool(name="tps2", bufs=2, space="PSUM") as t_ps):
                  edge_layer(win_ps, t_ps, fs2_t, fd2_t, 1, h2T_own)

              # ---- phase 6: output projection ----
              if max_phase >= 6:
                with tc.tile_pool(name="ps3", bufs=2, space="PSUM") as psp:
                  for c0 in range(0, NB, 512):
                      cn = min(512, NB - c0)
                      ps = psp.tile([2, 512], F32, tag="out_psum", space="PSUM")
                      nc.tensor.matmul(out=ps[:, 0:cn], lhsT=wout_sb[:],
                                       rhs=h2T_own[:, c0:c0 + cn],
                                       start=True, stop=True)
                      ob = wpool.tile([2, 512], F32, tag="out_sb")
                      nc.vector.tensor_scalar_add(out=ob[:, 0:cn],
                                                  in0=ps[:, 0:cn],
                                                  scalar1=bout_sb[:, :])
                      nc.sync.dma_start(out=outT_d[:, c0:c0 + cn],
                                        in_=ob[:, 0:cn])

    nc.compile()
    return nc


def _prepare(src, dst):
    if "prog" not in _CACHE:
        sched, fs_idx, fd_idx, dstw = _prep_edges(src, dst)
        nc = _build_program(sched)
        _CACHE["prog"] = (nc, fs_idx, fd_idx, dstw)
    return _CACHE["prog"]


def make_in_maps(feature, src, dst, W_in, b_in, fc_src_W, fc_src_b,
                 fc_dst_W, fc_dst_b, attn, W_out, b_out):
    nc, fs_idx, fd_idx, dstw = _prepare(src, dst)
    feature = np.asarray(feature, np.float32)
    W_in = np.asarray(W_in, np.float32)
    b_in = np.asarray(b_in, np.float32)
    fc_src_W = np.asarray(fc_src_W, np.float32)
    fc_src_b = np.asarray(fc_src_b, np.float32)
    fc_dst_W = np.asarray(fc_dst_W, np.float32)
    fc_dst_b = np.asarray(fc_dst_b, np.float32)
    attn = np.asarray(attn, np.float32)
    W_out = np.asarray(W_out, np.float32)
    b_out = np.asarray(b_out, np.float32)

    wfs1 = (W_in @ fc_src_W[0]).astype(BF16)
    wfd1 = (W_in @ fc_dst_W[0]).astype(BF16)
    bfs1 = b_in @ fc_src_W[0] + fc_src_b[0]
    bfd1 = b_in @ fc_dst_W[0] + fc_dst_b[0]
    bias = np.stack([bfs1, bfd1, fc_src_b[1], fc_dst_b[1]])
    bias_rep = np.tile(bias[None], (P, 1, 1)).astype(BF16)
    arep = np.tile(attn.reshape(2, HF)[None], (P, 1, 1)).astype(BF16)
    iota_rep = np.tile(np.arange(P, dtype=np.float32), (P, MAXJ)).astype(BF16)
    ident = np.eye(P, dtype=np.float32).astype(BF16)
    featT = np.ascontiguousarray(feature.T).astype(BF16)

    common = {
        "featT": featT, "wfs1": wfs1, "wfd1": wfd1,
        "ws2": fc_src_W[1].astype(BF16), "wd2": fc_dst_W[1].astype(BF16),
        "bias": bias_rep, "arep": arep, "iota_rep": iota_rep, "ident": ident,
        "wout": W_out.astype(BF16),
        "bout": b_out.reshape(2, 1).astype(np.float32),
    }
    in_maps = []
    for c in range(N_CORES):
        m = dict(common)
        m["featT_own"] = np.ascontiguousarray(featT[:, c * NB:(c + 1) * NB])
        m["fs_idx"] = fs_idx[c]
        m["fd_idx"] = fd_idx[c]
        m["dstw"] = dstw[c].astype(BF16)
        in_maps.append(m)
    return nc, in_maps


def kernel(feature, src, dst, W_in, b_in, fc_src_W, fc_src_b,
           fc_dst_W, fc_dst_b, attn, W_out, b_out):
    from concourse import bass_utils

    nc, in_maps = make_in_maps(feature, src, dst, W_in, b_in, fc_src_W,
                               fc_src_b, fc_dst_W, fc_dst_b, attn, W_out,
                               b_out)
    res = bass_utils.run_bass_kernel_spmd(nc, in_maps,
                                          core_ids=list(range(N_CORES)))
    out = np.concatenate(
        [res.results[c]["outT"].T for c in range(N_CORES)], axis=0)
    return out.astype(np.float32)



# revision 17
# speedup vs baseline: 1.3216x; 1.3216x over previous
"""Trainium2 Bass kernel for 2-layer GATv2 (N=50000, E=800000, 128->64->64->2).

Strategy (edge-parallel, dst-sharded, 8 NeuronCores):
  * Host sorts edges by dst; core c owns dst nodes [c*N/8, (c+1)*N/8).
  * The softmax denominator factors out of the weighted sum, so each layer is
    ONE edge pass: gather fs[src], fd[dst]; score = a . lrelu(fs+fd);
    e = exp(score) (max-subtraction skipped -- scores are O(1)); a 0/1
    selection-matrix matmul scatter-adds [e*fs[src] | e] into per-128-node
    window PSUM accumulators; h = relu(u/s).
  * fs tables are per-core-replicated (src is global); fd tables are local.
  * dma_gather (Q7 SWDGE, int16 idx): fs indices split lo/hi at 32768; edges
    within each window group are reordered lo-first (sums are order-invariant).
  * Between layers: AllGather of h1^T pieces (ncfw collective).
"""
import sys
import numpy as np

sys.path.insert(0, "/opt/trn_rl_repo")

import ml_dtypes

BF16 = ml_dtypes.bfloat16

# ---------------- problem constants (hardcoded per contract) ----------------
N = 50000
E = 800000
IN_F = 128
HF = 64          # hidden feats
HEADS = 4
DH = 16
NEG_SLOPE = 0.2
N_CORES = 8
NB = N // N_CORES            # nodes per core
WIN = 128                    # window size (nodes)
WPC = (NB + WIN - 1) // WIN  # windows per core
GRP = 4                      # windows per psum group
ST = 16                      # tiles per DVE supertile
LO_SPLIT = 25000             # fs index split (balances q0/q1 descriptor load)
P = 128

_CACHE = {}
MAX_PHASE = 6
JUNK_SAFE = False
USE_PRELU = True  # cayman exp_and_others table holds Exp+Prelu+Copy+Relu together
EDGE_LEVEL = 3
REPEAT = 1
MAXJ = 32  # max scatter jobs per supertile chunk (batched sel build width)


def _wrap16(vals):
    """int array [n] (n % 16 == 0) -> [128, n/16] int16 wrapped+replicated."""
    b = vals.reshape(-1, 16).T.astype(np.int16)
    return np.tile(b, (8, 1))


def _prep_edges(src, dst):
    """Sort by dst, shard by dst range, group-level lo/hi packing.

    Stream per group of GRP windows: [lo(w0)..lo(w3) | hi(w0)..hi(w3)], padded
    to 128-edge tiles only at the lo/hi block level. A tile may span several
    windows; the per-(tile,window) scatter matmuls are emitted as "jobs" with
    their own dst-rel column (-1 outside the window).
    """
    src = np.asarray(src, dtype=np.int64)
    dst = np.asarray(dst, dtype=np.int64)
    perm = np.argsort(dst, kind="stable")
    se, de = src[perm], dst[perm]
    per_cw = [[None] * WPC for _ in range(N_CORES)]
    for c in range(N_CORES):
        a = np.searchsorted(de, c * NB, side="left")
        b = np.searchsorted(de, (c + 1) * NB, side="left")
        s_c, r_c = se[a:b], de[a:b] - c * NB
        w_c = r_c // WIN
        for w in range(WPC):
            m = w_c == w
            s_w, r_w = s_c[m], r_c[m]
            lo = s_w < LO_SPLIT
            per_cw[c][w] = (s_w[lo], r_w[lo], s_w[~lo], r_w[~lo])

    groups = []
    for g0 in range(0, WPC, GRP):
        ws = list(range(g0, min(g0 + GRP, WPC)))
        # per-core per-seg edge counts -> group tile counts (max over cores)
        lo_tot = [sum(len(per_cw[c][w][0]) for w in ws) for c in range(N_CORES)]
        hi_tot = [sum(len(per_cw[c][w][2]) for w in ws) for c in range(N_CORES)]
        T_lo = max(-(-n // P) for n in lo_tot)
        T_hi = max(-(-n // P) for n in hi_tot)
        gt = T_lo + T_hi
        # jobs: union over cores of (tile, w) touched
        jobs_set = {}
        for c in range(N_CORES):
            pos = 0
            for seg, base in ((0, 0), (2, T_lo * P)):
                pos = base
                for w in ws:
                    n = len(per_cw[c][w][seg])
                    if n:
                        for t in range(pos // P, -(-(pos + n) // P)):
                            jobs_set[(t, w)] = True
                    pos += n
        jobs = sorted(jobs_set.keys())
        first_j, last_j = {}, {}
        for j, (t, w) in enumerate(jobs):
            if w not in first_j:
                first_j[w] = j
            last_j[w] = j
        groups.append({"ws": ws, "gt": gt, "T_lo": T_lo, "T_hi": T_hi,
                       "jobs": jobs, "first_j": first_j, "last_j": last_j})
    TT = sum(g["gt"] for g in groups)
    NJ = sum(len(g["jobs"]) for g in groups)

    fs_idx = np.zeros((N_CORES, P, TT * 8), np.int16)
    fd_idx = np.zeros((N_CORES, P, TT * 8), np.int16)
    dstw = np.full((N_CORES, P, NJ), -1.0, np.float32)
    for c in range(N_CORES):
        col = 0
        j_base = 0
        for g in groups:
            gt, T_lo = g["gt"], g["T_lo"]
            s_all = np.zeros(gt * P, np.int64)
            d_all = np.zeros(gt * P, np.int64)
            wof = np.full(gt * P, -1, np.int64)   # window of each slot
            r_all = np.zeros(gt * P, np.int64)
            for seg, base in ((0, 0), (2, T_lo * P)):
                pos = base
                for w in g["ws"]:
                    s_w = per_cw[c][w][seg]
                    r_w = per_cw[c][w][seg + 1]
                    n = len(s_w)
                    s_all[pos:pos + n] = s_w - (LO_SPLIT if seg else 0)
                    d_all[pos:pos + n] = r_w
                    r_all[pos:pos + n] = r_w
                    wof[pos:pos + n] = w
                    pos += n
            fs_idx[c, :, col:col + T_lo * 8] = _wrap16(s_all[:T_lo * P])
            if gt - T_lo:
                fs_idx[c, :, col + T_lo * 8:col + gt * 8] = \
                    _wrap16(s_all[T_lo * P:])
            fd_idx[c, :, col:col + gt * 8] = _wrap16(d_all)
            col += gt * 8
            # dstw per job
            for j, (t, w) in enumerate(g["jobs"]):
                sl = slice(t * P, (t + 1) * P)
                v = np.where(wof[sl] == w, r_all[sl] - w * WIN, -1.0)
                dstw[c, :, j_base + j] = v
            j_base += len(g["jobs"])
    return {"groups": groups, "TT": TT, "NJ": NJ}, fs_idx, fd_idx, dstw


def _build_program(sched):
    import concourse.bacc as bacc
    import concourse.mybir as mybir
    import concourse.tile as tile

    BF = mybir.dt.bfloat16
    F32 = mybir.dt.float32
    I16 = mybir.dt.int16
    AF = mybir.ActivationFunctionType
    OP = mybir.AluOpType
    AX = mybir.AxisListType

    TT = sched["TT"]
    NJ = sched["NJ"]
    groups = sched["groups"]

    nc = bacc.Bacc("TRN2", target_bir_lowering=False, debug=False,
                   num_devices=N_CORES, num_swdge_queues=4)

    featT = nc.dram_tensor("featT", [IN_F, N], BF, kind="ExternalInput").ap()
    featT_own = nc.dram_tensor("featT_own", [IN_F, NB], BF,
                               kind="ExternalInput").ap()
    fs_idx_d = nc.dram_tensor("fs_idx", [P, TT * 8], I16,
                              kind="ExternalInput").ap()
    fd_idx_d = nc.dram_tensor("fd_idx", [P, TT * 8], I16,
                              kind="ExternalInput").ap()
    dstw_d = nc.dram_tensor("dstw", [P, NJ], BF, kind="ExternalInput").ap()
    wfs1_d = nc.dram_tensor("wfs1", [IN_F, HF], BF, kind="ExternalInput").ap()
    wfd1_d = nc.dram_tensor("wfd1", [IN_F, HF], BF, kind="ExternalInput").ap()
    ws2_d = nc.dram_tensor("ws2", [HF, HF], BF, kind="ExternalInput").ap()
    wd2_d = nc.dram_tensor("wd2", [HF, HF], BF, kind="ExternalInput").ap()
    bias_d = nc.dram_tensor("bias", [P, 4, HF], BF, kind="ExternalInput").ap()
    arep_d = nc.dram_tensor("arep", [P, 2, HF], BF, kind="ExternalInput").ap()
    iota_rep_d = nc.dram_tensor("iota_rep", [P, MAXJ * P], BF,
                                kind="ExternalInput").ap()
    ident_d = nc.dram_tensor("ident", [P, P], BF, kind="ExternalInput").ap()
    wout_d = nc.dram_tensor("wout", [HF, 2], BF, kind="ExternalInput").ap()
    bout_d = nc.dram_tensor("bout", [2, 1], F32, kind="ExternalInput").ap()
    outT_d = nc.dram_tensor("outT", [2, NB], F32, kind="ExternalOutput").ap()

    fs1_t = nc.dram_tensor("fs1_t", [N, P], BF).ap()   # cols 0:64 live
    fd1_t = nc.dram_tensor("fd1_t", [NB, P], BF).ap()
    fs2_own = nc.dram_tensor("fs2_own", [NB, P], BF).ap()
    fs2_t = nc.dram_tensor("fs2_t", [N, P], BF, addr_space="Shared").ap()
    fd2_t = nc.dram_tensor("fd2_t", [NB, P], BF).ap()

    with tile.TileContext(nc) as tc:
        with (
            tc.tile_pool(name="const", bufs=1) as cpool,
            tc.tile_pool(name="work", bufs=2) as wpool,
            tc.tile_pool(name="gath", bufs=3) as gpool,
        ):
            def cload(name, shape, dt_, src_ap):
                t = cpool.tile(shape, dt_, tag=name)
                nc.sync.dma_start(out=t[:], in_=src_ap)
                return t

            dstw_sb = cload("dstw_sb", [P, NJ], BF, dstw_d[:, :])
            wfs1_sb = cload("wfs1_sb", [IN_F, HF], BF, wfs1_d[:, :])
            wfd1_sb = cload("wfd1_sb", [IN_F, HF], BF, wfd1_d[:, :])
            ws2_sb = cload("ws2_sb", [HF, HF], BF, ws2_d[:, :])
            wd2_sb = cload("wd2_sb", [HF, HF], BF, wd2_d[:, :])
            bias_sb = cload("bias_sb", [P, 4, HF], BF, bias_d[:, :, :])
            arep_sb = cload("arep_sb", [P, 2, HF], BF, arep_d[:, :, :])
            iota_rep_sb = cload("iota_rep_sb", [P, MAXJ * P], BF,
                                iota_rep_d[:, :])
            ident_sb = cload("ident_sb", [P, P], BF, ident_d[:, :])
            wout_sb = cload("wout_sb", [HF, 2], BF, wout_d[:, :])
            bout_sb = cload("bout_sb", [2, 1], F32, bout_d[:, :])
            h1T_own = cpool.tile([HF, NB], BF, tag="h1T_own")
            h2T_own = cpool.tile([HF, NB], BF, tag="h2T_own")

            def project(psp, dst_table, n_rows, row0, lhsT_of, w_sb, bias_idx):
                """dst_table[row0+i, 0:64] = lhsT(i)^T @ w + bias (batches)."""
                BATCH = 8 * P
                for b0 in range(0, n_rows, BATCH):
                    bn = min(BATCH, n_rows - b0)
                    nch = -(-bn // P)
                    ps = psp.tile([P, 8 * HF], F32, tag="proj_psum",
                                  space="PSUM")
                    for k in range(nch):
                        c0 = b0 + k * P
                        cn = min(P, n_rows - c0)
                        nc.tensor.matmul(
                            out=ps[0:cn, k * HF:(k + 1) * HF],
                            lhsT=lhsT_of(c0, cn), rhs=w_sb[:],
                            start=True, stop=True)
                    ob = wpool.tile([P, 8, P], BF, tag="proj_out")
                    if JUNK_SAFE:
                        nc.vector.memset(ob[:, :, HF:P], 0.0)
                    wcols = P if JUNK_SAFE else HF
                    nc.vector.tensor_add(
                        out=ob[:, 0:nch, 0:HF],
                        in0=ps[:].rearrange("p (k f) -> p k f", k=8)[:, 0:nch, :],
                        in1=bias_sb[:, bias_idx, :].unsqueeze(1)
                            .to_broadcast([P, nch, HF]))
                    nf = bn // P
                    if nf:
                        nc.sync.dma_start(
                            out=dst_table[row0 + b0:row0 + b0 + nf * P, 0:wcols]
                                .rearrange("(k p) f -> p k f", p=P),
                            in_=ob[:, 0:nf, 0:wcols])
                    if bn - nf * P:
                        nc.sync.dma_start(
                            out=dst_table[row0 + b0 + nf * P:row0 + b0 + bn,
                                          0:wcols],
                            in_=ob[0:bn - nf * P, nf, 0:wcols])

            def edge_layer(win_ps, hT_ps_pool, fs_table, fd_table, a_idx,
                           hT_own):
                t_base = 0
                col = 0
                j_base = 0
                for g in groups:
                    gt = g["gt"]
                    n_lo = g["T_lo"]
                    fsg = gpool.tile([P, gt, P], BF, tag="fsg")
                    fdg = gpool.tile([P, gt, P], BF, tag="fdg")
                    fs_ix = gpool.tile([P, gt * 8], I16, tag="fs_ix")
                    nc.sync.dma_start(out=fs_ix[:],
                                      in_=fs_idx_d[:, col:col + gt * 8])
                    fd_ix = gpool.tile([P, gt * 8], I16, tag="fd_ix")
                    nc.sync.dma_start(out=fd_ix[:],
                                      in_=fd_idx_d[:, col:col + gt * 8])
                    gt2 = gt // 2
                    if n_lo and EDGE_LEVEL >= 0:
                        nc.gpsimd.dma_gather(
                            fsg[:, 0:n_lo, :], fs_table[0:LO_SPLIT, :],
                            fs_ix[:, 0:n_lo * 8], n_lo * P, n_lo * P, P,
                            single_packet=False, queue_num=0)
                    if gt - n_lo and EDGE_LEVEL >= 0:
                        nc.gpsimd.dma_gather(
                            fsg[:, n_lo:gt, :], fs_table[LO_SPLIT:N, :],
                            fs_ix[:, n_lo * 8:gt * 8],
                            (gt - n_lo) * P, (gt - n_lo) * P, P,
                            single_packet=False, queue_num=1)
                    if gt2 and EDGE_LEVEL >= 0:
                        nc.gpsimd.dma_gather(
                            fdg[:, 0:gt2, :], fd_table[:, :],
                            fd_ix[:, 0:gt2 * 8], gt2 * P, gt2 * P, P,
                            single_packet=False, queue_num=2)
                    if gt - gt2 and EDGE_LEVEL >= 0:
                        nc.gpsimd.dma_gather(
                            fdg[:, gt2:gt, :], fd_table[:, :],
                            fd_ix[:, gt2 * 8:gt * 8],
                            (gt - gt2) * P, (gt - gt2) * P, P,
                            single_packet=False, queue_num=3)
                    col += gt * 8

                    jobs = g["jobs"]
                    first_j, last_j = g["first_j"], g["last_j"]
                    psums = {w: win_ps.tile([P, HF + HEADS], F32, name="win_psum",
                                            tag="win_psum", space="PSUM")
                             for w in first_j}

                    for s0 in range(0, gt, ST):
                        if EDGE_LEVEL < 1:
                            break
                        sn = min(ST, gt - s0)
                        fs_v = fsg[:, s0:s0 + sn, 0:HF]
                        fd_v = fdg[:, s0:s0 + sn, 0:HF]
                        t0 = wpool.tile([P, ST, HF], BF, tag="t0")
                        nc.vector.tensor_add(out=t0[:, 0:sn, :], in0=fs_v,
                                             in1=fd_v)
                        t1 = wpool.tile([P, ST, HF], BF, tag="t1")
                        if USE_PRELU:
                            nc.scalar.activation(
                                out=t1[:, 0:sn, :], in_=t0[:, 0:sn, :],
                                func=AF.Prelu, alpha=NEG_SLOPE)
                        else:
                            nc.vector.scalar_tensor_tensor(
                                out=t1[:, 0:sn, :], in0=t0[:, 0:sn, :],
                                scalar=NEG_SLOPE, in1=t0[:, 0:sn, :],
                                op0=OP.mult, op1=OP.max)
                        t2 = wpool.tile([P, ST, HF], BF, tag="t2")
                        nc.vector.tensor_mul(
                            out=t2[:, 0:sn, :], in0=t1[:, 0:sn, :],
                            in1=arep_sb[:, a_idx, :].unsqueeze(1)
                                .to_broadcast([P, sn, HF]))
                        t3 = wpool.tile([P, ST, HEADS, DH // 2], BF,
                                        tag="t3")
                        t2v = t2[:, 0:sn, :].rearrange(
                            "p t (h d) -> p (t h) d", d=DH)
                        nc.vector.tensor_add(
                            out=t3[:, 0:sn, :, :]
                                .rearrange("p t h d -> p (t h) d"),
                            in0=t2v[:, :, 0:DH // 2],
                            in1=t2v[:, :, DH // 2:DH])
                        sc = wpool.tile([P, ST * HEADS], F32, tag="sc")
                        nc.vector.tensor_reduce(
                            out=sc[:, 0:sn * HEADS]
                                .rearrange("p (t h) -> p t h", h=HEADS),
                            in_=t3[:, 0:sn, :, :]
                                .rearrange("p t h d -> p (t h) d"),
                            op=OP.add, axis=AX.X)
                        rhs = wpool.tile([P, ST, HF + HEADS], BF, tag="rhs")
                        nc.scalar.activation(
                            out=rhs[:, 0:sn, HF:HF + HEADS],
                            in_=sc[:, 0:sn * HEADS]
                                .rearrange("p (t h) -> p t h", h=HEADS),
                            func=AF.Exp)
                        nc.vector.tensor_mul(
                            out=rhs[:, 0:sn, 0:HF]
                                .rearrange("p t (h d) -> p t h d", d=DH),
                            in0=fs_v.rearrange("p t (h d) -> p t h d", d=DH),
                            in1=rhs[:, 0:sn, HF:HF + HEADS].unsqueeze(3)
                                .to_broadcast([P, sn, HEADS, DH]))
                        chunk_jobs = [(j, t, w) for j, (t, w) in
                                      enumerate(jobs) if s0 <= t < s0 + sn]
                        nJ = len(chunk_jobs)
                        assert nJ <= MAXJ, (nJ, MAXJ)
                        sel = wpool.tile([P, MAXJ, P], BF, tag="sel")
                        if EDGE_LEVEL >= 2 and nJ:
                            j0c = chunk_jobs[0][0]
                            nc.vector.tensor_tensor(
                                out=sel[:, 0:nJ, :],
                                in0=iota_rep_sb[:, 0:nJ * P]
                                    .rearrange("p (j n) -> p j n", n=P),
                                in1=dstw_sb[:, j_base + j0c:
                                            j_base + j0c + nJ]
                                    .unsqueeze(2).to_broadcast([P, nJ, P]),
                                op=OP.is_equal)
                            for js, (j, t, w) in enumerate(chunk_jobs):
                                nc.tensor.matmul(
                                    out=psums[w][:], lhsT=sel[:, js, :],
                                    rhs=rhs[:, t - s0, :],
                                    start=(j == first_j[w]),
                                    stop=(j == last_j[w]))

                    for w in g["ws"]:
                        if w not in first_j or EDGE_LEVEL < 2:
                            continue
                        ps = psums[w]
                        nw = min(WIN, NB - w * WIN)
                        s_eps = wpool.tile([P, HEADS], F32, tag="s_eps")
                        nc.vector.tensor_scalar_add(
                            out=s_eps[:], in0=ps[:, HF:HF + HEADS],
                            scalar1=1e-20)
                        s_inv = wpool.tile([P, HEADS], F32, tag="s_inv")
                        nc.vector.reciprocal(out=s_inv[:], in_=s_eps[:])
                        hw_ = wpool.tile([P, HF], BF, tag="hw_")
                        nc.vector.tensor_mul(
                            out=hw_[:].rearrange("p (h d) -> p h d", d=DH),
                            in0=ps[:, 0:HF].rearrange("p (h d) -> p h d",
                                                      d=DH),
                            in1=s_inv[:].unsqueeze(2)
                                .to_broadcast([P, HEADS, DH]))
                        hrel = wpool.tile([P, HF], BF, tag="hrel")
                        nc.scalar.activation(out=hrel[:], in_=hw_[:],
                                             func=AF.Relu)
                        if EDGE_LEVEL < 3:
                            continue
                        hT_ps = hT_ps_pool.tile([HF, P], BF, tag="hT_ps",
                                                space="PSUM")
                        nc.tensor.transpose(out=hT_ps[:], in_=hrel[:],
                                            identity=ident_sb[:])
                        nc.vector.tensor_copy(
                            out=hT_own[:, w * WIN:w * WIN + nw],
                            in_=hT_ps[:, 0:nw])
                    t_base += gt
                    j_base += len(jobs)

            def batched_lhsT(src_ap, width, tag):
                cache = {}

                def f(c0, cn):
                    b0 = (c0 // (8 * P)) * (8 * P)
                    if cache.get("b0") != b0:
                        bw = min(8 * P, width - b0)
                        t = wpool.tile([src_ap.shape[0], 8 * P], BF, tag=tag)
                        nc.sync.dma_start(out=t[:, 0:bw],
                                          in_=src_ap[:, b0:b0 + bw])
                        cache["b0"], cache["t"] = b0, t
                    return cache["t"][:, c0 - b0:c0 - b0 + cn]
                return f

            # ---- phase 1: layer-1 projections ----
            max_phase = MAX_PHASE
            for _rep in range(REPEAT):
              with tc.tile_pool(name="ps1", bufs=2, space="PSUM") as psp:
                  project(psp, fd1_t, NB, 0,
                          batched_lhsT(featT_own, NB, "featT_own_chunk"),
                          wfd1_sb, 1)
                  project(psp, fs1_t, N, 0,
                          batched_lhsT(featT, N, "featT_chunk"), wfs1_sb, 0)

              # ---- phase 2: layer-1 edge pass ----
              if max_phase >= 2:
                with (tc.tile_pool(name="wps1", bufs=6, space="PSUM") as win_ps,
                    tc.tile_pool(name="tps1", bufs=2, space="PSUM") as t_ps):
                  edge_layer(win_ps, t_ps, fs1_t, fd1_t, 0, h1T_own)

              # ---- phase 3+4: layer-2 projections (own rows) + AllGather ----
              if max_phase >= 3:
                with tc.tile_pool(name="ps2", bufs=2, space="PSUM") as psp:
                  project(psp, fd2_t, NB, 0,
                          lambda c0, cn: h1T_own[:, c0:c0 + cn], wd2_sb, 3)
                  project(psp, fs2_own, NB, 0,
                          lambda c0, cn: h1T_own[:, c0:c0 + cn], ws2_sb, 2)
                nc.gpsimd.collective_compute(
                  "AllGather", OP.bypass, ins=[fs2_own[:, :]],
                  outs=[fs2_t[:, :]],
                  replica_groups=[list(range(N_CORES))])

              # ---- phase 5: layer-2 edge pass ----
              if max_phase >= 4:
                with (tc.tile_pool(name="wps2", bufs=6, space="PSUM") as win_ps,
                    tc.tile_pool(name="tps2", bufs=2, space="PSUM") as t_ps):
                  edge_layer(win_ps, t_ps, fs2_t, fd2_t, 1, h2T_own)

              # ---- phase 6: output projection ----
              if max_phase >= 6:
                with tc.tile_pool(name="ps3", bufs=2, space="PSUM") as psp:
                  for c0 in range(0, NB, 512):
                      cn = min(512, NB - c0)
                      ps = psp.tile([2, 512], F32, tag="out_psum", space="PSUM")
                      nc.tensor.matmul(out=ps[:, 0:cn], lhsT=wout_sb[:],
                                       rhs=h2T_own[:, c0:c0 + cn],
                                       start=True, stop=True)
                      ob = wpool.tile([2, 512], F32, tag="out_sb")
                      nc.vector.tensor_scalar_add(out=ob[:, 0:cn],
                                                  in0=ps[:, 0:cn],
                                                  scalar1=bout_sb[:, :])
                      nc.sync.dma_start(out=outT_d[:, c0:c0 + cn],
                                        in_=ob[:, 0:cn])

    nc.compile()
    return nc


def _prepare(src, dst):
    if "prog" not in _CACHE:
        sched, fs_idx, fd_idx, dstw = _prep_edges(src, dst)
        nc = _build_program(sched)
        _CACHE["prog"] = (nc, fs_idx, fd_idx, dstw)
    return _CACHE["prog"]


def make_in_maps(feature, src, dst, W_in, b_in, fc_src_W, fc_src_b,
                 fc_dst_W, fc_dst_b, attn, W_out, b_out):
    nc, fs_idx, fd_idx, dstw = _prepare(src, dst)
    feature = np.asarray(feature, np.float32)
    W_in = np.asarray(W_in, np.float32)
    b_in = np.asarray(b_in, np.float32)
    fc_src_W = np.asarray(fc_src_W, np.float32)
    fc_src_b = np.asarray(fc_src_b, np.float32)
    fc_dst_W = np.asarray(fc_dst_W, np.float32)
    fc_dst_b = np.asarray(fc_dst_b, np.float32)
    attn = np.asarray(attn, np.float32)
    W_out = np.asarray(W_out, np.float32)
    b_out = np.asarray(b_out, np.float32)

    wfs1 = (W_in @ fc_src_W[0]).astype(BF16)
    wfd1 = (W_in @ fc_dst_W[0]).astype(BF16)
    bfs1 = b_in @ fc_src_W[0] + fc_src_b[0]
    bfd1 = b_in @ fc_dst_W[0] + fc_dst_b[0]
    bias = np.stack([bfs1, bfd1, fc_src_b[1], fc_dst_b[1]])
    bias_rep = np.tile(bias[None], (P, 1, 1)).astype(BF16)
    arep = np.tile(attn.reshape(2, HF)[None], (P, 1, 1)).astype(BF16)
    iota_rep = np.tile(np.arange(P, dtype=np.float32), (P, MAXJ)).astype(BF16)
    ident = np.eye(P, dtype=np.float32).astype(BF16)
    featT = np.ascontiguousarray(feature.T).astype(BF16)

    common = {
        "featT": featT, "wfs1": wfs1, "wfd1": wfd1,
        "ws2": fc_src_W[1].astype(BF16), "wd2": fc_dst_W[1].astype(BF16),
        "bias": bias_rep, "arep": arep, "iota_rep": iota_rep, "ident": ident,
        "wout": W_out.astype(BF16),
        "bout": b_out.reshape(2, 1).astype(np.float32),
    }
    in_maps = []
    for c in range(N_CORES):
        m = dict(common)
        m["featT_own"] = np.ascontiguousarray(featT[:, c * NB:(c + 1) * NB])
        m["fs_idx"] = fs_idx[c]
        m["fd_idx"] = fd_idx[c]
        m["dstw"] = dstw[c].astype(BF16)
        in_maps.append(m)
    return nc, in_maps


def kernel(feature, src, dst, W_in, b_in, fc_src_W, fc_src_b,
           fc_dst_W, fc_dst_b, attn, W_out, b_out):
    from concourse import bass_utils

    nc, in_maps = make_in_maps(feature, src, dst, W_in, b_in, fc_src_W,
                               fc_src_b, fc_dst_W, fc_dst_b, attn, W_out,
                               b_out)
    res = bass_utils.run_bass_kernel_spmd(nc, in_maps,
                                          core_ids=list(range(N_CORES)))
    out = np.concatenate(
        [res.results[c]["outT"].T for c in range(N_CORES)], axis=0)
    return out.astype(np.float32)



# revision 18
# speedup vs baseline: 1.3694x; 1.0362x over previous
"""Trainium2 Bass kernel for 2-layer GATv2 (N=50000, E=800000, 128->64->64->2).

Strategy (edge-parallel, dst-sharded, 8 NeuronCores):
  * Host sorts edges by dst; core c owns dst nodes [c*N/8, (c+1)*N/8).
  * The softmax denominator factors out of the weighted sum, so each layer is
    ONE edge pass: gather fs[src], fd[dst]; score = a . lrelu(fs+fd);
    e = exp(score) (max-subtraction skipped -- scores are O(1)); a 0/1
    selection-matrix matmul scatter-adds [e*fs[src] | e] into per-128-node
    window PSUM accumulators; h = relu(u/s).
  * fs tables are per-core-replicated (src is global); fd tables are local.
  * dma_gather (Q7 SWDGE, int16 idx): fs indices split lo/hi at 32768; edges
    within each window group are reordered lo-first (sums are order-invariant).
  * Between layers: AllGather of h1^T pieces (ncfw collective).
"""
import sys
import numpy as np

sys.path.insert(0, "/opt/trn_rl_repo")

import ml_dtypes

BF16 = ml_dtypes.bfloat16

# ---------------- problem constants (hardcoded per contract) ----------------
N = 50000
E = 800000
IN_F = 128
HF = 64          # hidden feats
HEADS = 4
DH = 16
NEG_SLOPE = 0.2
N_CORES = 8
NB = N // N_CORES            # nodes per core
WIN = 128                    # window size (nodes)
WPC = (NB + WIN - 1) // WIN  # windows per core
GRP = 4                      # windows per psum group
ST = 16                      # tiles per DVE supertile
LO_SPLIT = 25000             # fs index split (balances q0/q1 descriptor load)
P = 128

_CACHE = {}
MAX_PHASE = 6
JUNK_SAFE = False
USE_PRELU = True  # cayman exp_and_others table holds Exp+Prelu+Copy+Relu together
EDGE_LEVEL = 3
REPEAT = 1
MAXJ = 32  # max scatter jobs per supertile chunk (batched sel build width)


def _wrap16(vals):
    """int array [n] (n % 16 == 0) -> [128, n/16] int16 wrapped+replicated."""
    b = vals.reshape(-1, 16).T.astype(np.int16)
    return np.tile(b, (8, 1))


def _prep_edges(src, dst):
    """Sort by dst, shard by dst range, group-level lo/hi packing.

    Stream per group of GRP windows: [lo(w0)..lo(w3) | hi(w0)..hi(w3)], padded
    to 128-edge tiles only at the lo/hi block level. A tile may span several
    windows; the per-(tile,window) scatter matmuls are emitted as "jobs" with
    their own dst-rel column (-1 outside the window).
    """
    src = np.asarray(src, dtype=np.int64)
    dst = np.asarray(dst, dtype=np.int64)
    perm = np.argsort(dst, kind="stable")
    se, de = src[perm], dst[perm]
    per_cw = [[None] * WPC for _ in range(N_CORES)]
    for c in range(N_CORES):
        a = np.searchsorted(de, c * NB, side="left")
        b = np.searchsorted(de, (c + 1) * NB, side="left")
        s_c, r_c = se[a:b], de[a:b] - c * NB
        w_c = r_c // WIN
        for w in range(WPC):
            m = w_c == w
            s_w, r_w = s_c[m], r_c[m]
            lo = s_w < LO_SPLIT
            per_cw[c][w] = (s_w[lo], r_w[lo], s_w[~lo], r_w[~lo])

    groups = []
    for g0 in range(0, WPC, GRP):
        ws = list(range(g0, min(g0 + GRP, WPC)))
        # per-core per-seg edge counts -> group tile counts (max over cores)
        lo_tot = [sum(len(per_cw[c][w][0]) for w in ws) for c in range(N_CORES)]
        hi_tot = [sum(len(per_cw[c][w][2]) for w in ws) for c in range(N_CORES)]
        T_lo = max(-(-n // P) for n in lo_tot)
        T_hi = max(-(-n // P) for n in hi_tot)
        gt = T_lo + T_hi
        # jobs: union over cores of (tile, w) touched
        jobs_set = {}
        for c in range(N_CORES):
            pos = 0
            for seg, base in ((0, 0), (2, T_lo * P)):
                pos = base
                for w in ws:
                    n = len(per_cw[c][w][seg])
                    if n:
                        for t in range(pos // P, -(-(pos + n) // P)):
                            jobs_set[(t, w)] = True
                    pos += n
        jobs = sorted(jobs_set.keys())
        first_j, last_j = {}, {}
        for j, (t, w) in enumerate(jobs):
            if w not in first_j:
                first_j[w] = j
            last_j[w] = j
        groups.append({"ws": ws, "gt": gt, "T_lo": T_lo, "T_hi": T_hi,
                       "jobs": jobs, "first_j": first_j, "last_j": last_j})
    TT = sum(g["gt"] for g in groups)
    NJ = sum(len(g["jobs"]) for g in groups)

    fs_idx = np.zeros((N_CORES, P, TT * 8), np.int16)
    fd_idx = np.zeros((N_CORES, P, TT * 8), np.int16)
    dstw = np.full((N_CORES, P, NJ), -1.0, np.float32)
    for c in range(N_CORES):
        col = 0
        j_base = 0
        for g in groups:
            gt, T_lo = g["gt"], g["T_lo"]
            s_all = np.zeros(gt * P, np.int64)
            d_all = np.zeros(gt * P, np.int64)
            wof = np.full(gt * P, -1, np.int64)   # window of each slot
            r_all = np.zeros(gt * P, np.int64)
            for seg, base in ((0, 0), (2, T_lo * P)):
                pos = base
                for w in g["ws"]:
                    s_w = per_cw[c][w][seg]
                    r_w = per_cw[c][w][seg + 1]
                    n = len(s_w)
                    s_all[pos:pos + n] = s_w - (LO_SPLIT if seg else 0)
                    d_all[pos:pos + n] = r_w
                    r_all[pos:pos + n] = r_w
                    wof[pos:pos + n] = w
                    pos += n
            fs_idx[c, :, col:col + T_lo * 8] = _wrap16(s_all[:T_lo * P])
            if gt - T_lo:
                fs_idx[c, :, col + T_lo * 8:col + gt * 8] = \
                    _wrap16(s_all[T_lo * P:])
            fd_idx[c, :, col:col + gt * 8] = _wrap16(d_all)
            col += gt * 8
            # dstw per job
            for j, (t, w) in enumerate(g["jobs"]):
                sl = slice(t * P, (t + 1) * P)
                v = np.where(wof[sl] == w, r_all[sl] - w * WIN, -1.0)
                dstw[c, :, j_base + j] = v
            j_base += len(g["jobs"])
    return {"groups": groups, "TT": TT, "NJ": NJ}, fs_idx, fd_idx, dstw


def _build_program(sched):
    import concourse.bacc as bacc
    import concourse.mybir as mybir
    import concourse.tile as tile

    BF = mybir.dt.bfloat16
    F32 = mybir.dt.float32
    I16 = mybir.dt.int16
    AF = mybir.ActivationFunctionType
    OP = mybir.AluOpType
    AX = mybir.AxisListType

    TT = sched["TT"]
    NJ = sched["NJ"]
    groups = sched["groups"]

    nc = bacc.Bacc("TRN2", target_bir_lowering=False, debug=False,
                   num_devices=N_CORES, num_swdge_queues=4)

    featT = nc.dram_tensor("featT", [IN_F, N], BF, kind="ExternalInput").ap()
    featT_own = nc.dram_tensor("featT_own", [IN_F, NB], BF,
                               kind="ExternalInput").ap()
    fs_idx_d = nc.dram_tensor("fs_idx", [P, TT * 8], I16,
                              kind="ExternalInput").ap()
    fd_idx_d = nc.dram_tensor("fd_idx", [P, TT * 8], I16,
                              kind="ExternalInput").ap()
    dstw_d = nc.dram_tensor("dstw", [P, NJ], BF, kind="ExternalInput").ap()
    wfs1_d = nc.dram_tensor("wfs1", [IN_F, HF], BF, kind="ExternalInput").ap()
    wfd1_d = nc.dram_tensor("wfd1", [IN_F, HF], BF, kind="ExternalInput").ap()
    ws2_d = nc.dram_tensor("ws2", [HF, HF], BF, kind="ExternalInput").ap()
    wd2_d = nc.dram_tensor("wd2", [HF, HF], BF, kind="ExternalInput").ap()
    bias_d = nc.dram_tensor("bias", [P, 4, HF], BF, kind="ExternalInput").ap()
    arep_d = nc.dram_tensor("arep", [P, 2, HF], BF, kind="ExternalInput").ap()
    iota_rep_d = nc.dram_tensor("iota_rep", [P, MAXJ * P], BF,
                                kind="ExternalInput").ap()
    ident_d = nc.dram_tensor("ident", [P, P], BF, kind="ExternalInput").ap()
    wout_d = nc.dram_tensor("wout", [HF, 2], BF, kind="ExternalInput").ap()
    bout_d = nc.dram_tensor("bout", [2, 1], F32, kind="ExternalInput").ap()
    outT_d = nc.dram_tensor("outT", [2, NB], F32, kind="ExternalOutput").ap()

    fs1_t = nc.dram_tensor("fs1_t", [N, P], BF).ap()   # cols 0:64 live
    fd1_t = nc.dram_tensor("fd1_t", [NB, P], BF).ap()
    fs2_own = nc.dram_tensor("fs2_own", [NB, P], BF).ap()
    fs2_t = nc.dram_tensor("fs2_t", [N, P], BF, addr_space="Shared").ap()
    fd2_t = nc.dram_tensor("fd2_t", [NB, P], BF).ap()

    with tile.TileContext(nc) as tc:
        with (
            tc.tile_pool(name="const", bufs=1) as cpool,
            tc.tile_pool(name="work", bufs=2) as wpool,
            tc.tile_pool(name="gath", bufs=3) as gpool,
        ):
            def cload(name, shape, dt_, src_ap):
                t = cpool.tile(shape, dt_, tag=name)
                nc.sync.dma_start(out=t[:], in_=src_ap)
                return t

            dstw_sb = cload("dstw_sb", [P, NJ], BF, dstw_d[:, :])
            wfs1_sb = cload("wfs1_sb", [IN_F, HF], BF, wfs1_d[:, :])
            wfd1_sb = cload("wfd1_sb", [IN_F, HF], BF, wfd1_d[:, :])
            ws2_sb = cload("ws2_sb", [HF, HF], BF, ws2_d[:, :])
            wd2_sb = cload("wd2_sb", [HF, HF], BF, wd2_d[:, :])
            bias_sb = cload("bias_sb", [P, 4, HF], BF, bias_d[:, :, :])
            arep_sb = cload("arep_sb", [P, 2, HF], BF, arep_d[:, :, :])
            iota_rep_sb = cload("iota_rep_sb", [P, MAXJ * P], BF,
                                iota_rep_d[:, :])
            ident_sb = cload("ident_sb", [P, P], BF, ident_d[:, :])
            wout_sb = cload("wout_sb", [HF, 2], BF, wout_d[:, :])
            bout_sb = cload("bout_sb", [2, 1], F32, bout_d[:, :])
            h1T_own = cpool.tile([HF, NB], BF, tag="h1T_own")
            h2T_own = cpool.tile([HF, NB], BF, tag="h2T_own")

            def project(psp, dst_table, n_rows, row0, lhsT_of, w_sb, bias_idx):
                """dst_table[row0+i, 0:64] = lhsT(i)^T @ w + bias (batches)."""
                BATCH = 8 * P
                for b0 in range(0, n_rows, BATCH):
                    bn = min(BATCH, n_rows - b0)
                    nch = -(-bn // P)
                    ps = psp.tile([P, 8 * HF], F32, tag="proj_psum",
                                  space="PSUM")
                    for k in range(nch):
                        c0 = b0 + k * P
                        cn = min(P, n_rows - c0)
                        nc.tensor.matmul(
                            out=ps[0:cn, k * HF:(k + 1) * HF],
                            lhsT=lhsT_of(c0, cn), rhs=w_sb[:],
                            start=True, stop=True)
                    ob = wpool.tile([P, 8, P], BF, tag="proj_out")
                    if JUNK_SAFE:
                        nc.vector.memset(ob[:, :, HF:P], 0.0)
                    wcols = P if JUNK_SAFE else HF
                    nc.vector.tensor_add(
                        out=ob[:, 0:nch, 0:HF],
                        in0=ps[:].rearrange("p (k f) -> p k f", k=8)[:, 0:nch, :],
                        in1=bias_sb[:, bias_idx, :].unsqueeze(1)
                            .to_broadcast([P, nch, HF]))
                    nf = bn // P
                    if nf:
                        nc.sync.dma_start(
                            out=dst_table[row0 + b0:row0 + b0 + nf * P, 0:wcols]
                                .rearrange("(k p) f -> p k f", p=P),
                            in_=ob[:, 0:nf, 0:wcols])
                    if bn - nf * P:
                        nc.sync.dma_start(
                            out=dst_table[row0 + b0 + nf * P:row0 + b0 + bn,
                                          0:wcols],
                            in_=ob[0:bn - nf * P, nf, 0:wcols])

            def edge_layer(win_ps, hT_ps_pool, fs_table, fd_table, a_idx,
                           hT_own):
                t_base = 0
                col = 0
                j_base = 0
                for g in groups:
                    gt = g["gt"]
                    n_lo = g["T_lo"]
                    fsg = gpool.tile([P, gt, P], BF, tag="fsg")
                    fdg = gpool.tile([P, gt, P], BF, tag="fdg")
                    fs_ix = gpool.tile([P, gt * 8], I16, tag="fs_ix")
                    nc.sync.dma_start(out=fs_ix[:],
                                      in_=fs_idx_d[:, col:col + gt * 8])
                    fd_ix = gpool.tile([P, gt * 8], I16, tag="fd_ix")
                    nc.sync.dma_start(out=fd_ix[:],
                                      in_=fd_idx_d[:, col:col + gt * 8])
                    gt2 = gt // 2
                    if n_lo and EDGE_LEVEL >= 0:
                        nc.gpsimd.dma_gather(
                            fsg[:, 0:n_lo, :], fs_table[0:LO_SPLIT, :],
                            fs_ix[:, 0:n_lo * 8], n_lo * P, n_lo * P, P,
                            single_packet=False, queue_num=0)
                    if gt - n_lo and EDGE_LEVEL >= 0:
                        nc.gpsimd.dma_gather(
                            fsg[:, n_lo:gt, :], fs_table[LO_SPLIT:N, :],
                            fs_ix[:, n_lo * 8:gt * 8],
                            (gt - n_lo) * P, (gt - n_lo) * P, P,
                            single_packet=False, queue_num=1)
                    if gt2 and EDGE_LEVEL >= 0:
                        nc.gpsimd.dma_gather(
                            fdg[:, 0:gt2, :], fd_table[:, :],
                            fd_ix[:, 0:gt2 * 8], gt2 * P, gt2 * P, P,
                            single_packet=False, queue_num=2)
                    if gt - gt2 and EDGE_LEVEL >= 0:
                        nc.gpsimd.dma_gather(
                            fdg[:, gt2:gt, :], fd_table[:, :],
                            fd_ix[:, gt2 * 8:gt * 8],
                            (gt - gt2) * P, (gt - gt2) * P, P,
                            single_packet=False, queue_num=3)
                    col += gt * 8

                    jobs = g["jobs"]
                    first_j, last_j = g["first_j"], g["last_j"]
                    psums = {w: win_ps.tile([P, HF + HEADS], F32, name="win_psum",
                                            tag="win_psum", space="PSUM")
                             for w in first_j}

                    for s0 in range(0, gt, ST):
                        if EDGE_LEVEL < 1:
                            break
                        sn = min(ST, gt - s0)
                        fs_v = fsg[:, s0:s0 + sn, 0:HF]
                        fd_v = fdg[:, s0:s0 + sn, 0:HF]
                        t0 = wpool.tile([P, ST, HF], BF, tag="t0")
                        nc.vector.tensor_add(out=t0[:, 0:sn, :], in0=fs_v,
                                             in1=fd_v)
                        t1 = wpool.tile([P, ST, HF], BF, tag="t1")
                        if USE_PRELU:
                            nc.scalar.activation(
                                out=t1[:, 0:sn, :], in_=t0[:, 0:sn, :],
                                func=AF.Prelu, alpha=NEG_SLOPE)
                        else:
                            nc.vector.scalar_tensor_tensor(
                                out=t1[:, 0:sn, :], in0=t0[:, 0:sn, :],
                                scalar=NEG_SLOPE, in1=t0[:, 0:sn, :],
                                op0=OP.mult, op1=OP.max)
                        t2 = wpool.tile([P, ST, HF], BF, tag="t2")
                        nc.vector.tensor_mul(
                            out=t2[:, 0:sn, :], in0=t1[:, 0:sn, :],
                            in1=arep_sb[:, a_idx, :].unsqueeze(1)
                                .to_broadcast([P, sn, HF]))
                        t3 = wpool.tile([P, ST, HEADS, DH // 2], BF,
                                        tag="t3")
                        t2v = t2[:, 0:sn, :].rearrange(
                            "p t (h d) -> p (t h) d", d=DH)
                        nc.vector.tensor_add(
                            out=t3[:, 0:sn, :, :]
                                .rearrange("p t h d -> p (t h) d"),
                            in0=t2v[:, :, 0:DH // 2],
                            in1=t2v[:, :, DH // 2:DH])
                        sc = wpool.tile([P, ST * HEADS], F32, tag="sc")
                        nc.vector.tensor_reduce(
                            out=sc[:, 0:sn * HEADS]
                                .rearrange("p (t h) -> p t h", h=HEADS),
                            in_=t3[:, 0:sn, :, :]
                                .rearrange("p t h d -> p (t h) d"),
                            op=OP.add, axis=AX.X)
                        rhs = wpool.tile([P, ST, HF + HEADS], BF, tag="rhs")
                        nc.scalar.activation(
                            out=rhs[:, 0:sn, HF:HF + HEADS],
                            in_=sc[:, 0:sn * HEADS]
                                .rearrange("p (t h) -> p t h", h=HEADS),
                            func=AF.Exp)
                        nc.vector.tensor_mul(
                            out=rhs[:, 0:sn, 0:HF]
                                .rearrange("p t (h d) -> p t h d", d=DH),
                            in0=fs_v.rearrange("p t (h d) -> p t h d", d=DH),
                            in1=rhs[:, 0:sn, HF:HF + HEADS].unsqueeze(3)
                                .to_broadcast([P, sn, HEADS, DH]))
                        chunk_jobs = [(j, t, w) for j, (t, w) in
                                      enumerate(jobs) if s0 <= t < s0 + sn]
                        nJ = len(chunk_jobs)
                        assert nJ <= MAXJ, (nJ, MAXJ)
                        sel = wpool.tile([P, MAXJ, P], BF, tag="sel")
                        if EDGE_LEVEL >= 2 and nJ:
                            j0c = chunk_jobs[0][0]
                            nc.vector.tensor_tensor(
                                out=sel[:, 0:nJ, :],
                                in0=iota_rep_sb[:, 0:nJ * P]
                                    .rearrange("p (j n) -> p j n", n=P),
                                in1=dstw_sb[:, j_base + j0c:
                                            j_base + j0c + nJ]
                                    .unsqueeze(2).to_broadcast([P, nJ, P]),
                                op=OP.is_equal)
                            for js, (j, t, w) in enumerate(chunk_jobs):
                                nc.tensor.matmul(
                                    out=psums[w][:], lhsT=sel[:, js, :],
                                    rhs=rhs[:, t - s0, :],
                                    start=(j == first_j[w]),
                                    stop=(j == last_j[w]))

                    for w in g["ws"]:
                        if w not in first_j or EDGE_LEVEL < 2:
                            continue
                        ps = psums[w]
                        nw = min(WIN, NB - w * WIN)
                        s_eps = wpool.tile([P, HEADS], F32, tag="s_eps")
                        nc.vector.tensor_scalar_add(
                            out=s_eps[:], in0=ps[:, HF:HF + HEADS],
                            scalar1=1e-20)
                        s_inv = wpool.tile([P, HEADS], F32, tag="s_inv")
                        nc.vector.reciprocal(out=s_inv[:], in_=s_eps[:])
                        hw_ = wpool.tile([P, HF], BF, tag="hw_")
                        nc.vector.tensor_mul(
                            out=hw_[:].rearrange("p (h d) -> p h d", d=DH),
                            in0=ps[:, 0:HF].rearrange("p (h d) -> p h d",
                                                      d=DH),
                            in1=s_inv[:].unsqueeze(2)
                                .to_broadcast([P, HEADS, DH]))
                        hrel = wpool.tile([P, HF], BF, tag="hrel")
                        nc.scalar.activation(out=hrel[:], in_=hw_[:],
                                             func=AF.Relu)
                        if EDGE_LEVEL < 3:
                            continue
                        hT_ps = hT_ps_pool.tile([HF, P], BF, tag="hT_ps",
                                                space="PSUM")
                        nc.tensor.transpose(out=hT_ps[:], in_=hrel[:],
                                            identity=ident_sb[:])
                        nc.vector.tensor_copy(
                            out=hT_own[:, w * WIN:w * WIN + nw],
                            in_=hT_ps[:, 0:nw])
                    t_base += gt
                    j_base += len(jobs)

            def batched_lhsT(src_ap, width, tag):
                cache = {}

                def f(c0, cn):
                    b0 = (c0 // (8 * P)) * (8 * P)
                    if cache.get("b0") != b0:
                        bw = min(8 * P, width - b0)
                        t = wpool.tile([src_ap.shape[0], 8 * P], BF, tag=tag)
                        nc.sync.dma_start(out=t[:, 0:bw],
                                          in_=src_ap[:, b0:b0 + bw])
                        cache["b0"], cache["t"] = b0, t
                    return cache["t"][:, c0 - b0:c0 - b0 + cn]
                return f

            # ---- phase 1: layer-1 projections ----
            max_phase = MAX_PHASE
            for _rep in range(REPEAT):
              with tc.tile_pool(name="ps1", bufs=2, space="PSUM") as psp:
                  project(psp, fs1_t, N, 0,
                          batched_lhsT(featT, N, "featT_chunk"), wfs1_sb, 0)
                  project(psp, fd1_t, NB, 0,
                          batched_lhsT(featT_own, NB, "featT_own_chunk"),
                          wfd1_sb, 1)

              # ---- phase 2: layer-1 edge pass ----
              if max_phase >= 2:
                with (tc.tile_pool(name="wps1", bufs=6, space="PSUM") as win_ps,
                    tc.tile_pool(name="tps1", bufs=2, space="PSUM") as t_ps):
                  edge_layer(win_ps, t_ps, fs1_t, fd1_t, 0, h1T_own)

              # ---- phase 3+4: layer-2 projections (own rows) + AllGather ----
              if max_phase >= 3:
                with tc.tile_pool(name="ps2", bufs=2, space="PSUM") as psp:
                  project(psp, fd2_t, NB, 0,
                          lambda c0, cn: h1T_own[:, c0:c0 + cn], wd2_sb, 3)
                  project(psp, fs2_own, NB, 0,
                          lambda c0, cn: h1T_own[:, c0:c0 + cn], ws2_sb, 2)
                nc.gpsimd.collective_compute(
                  "AllGather", OP.bypass, ins=[fs2_own[:, :]],
                  outs=[fs2_t[:, :]],
                  replica_groups=[list(range(N_CORES))])

              # ---- phase 5: layer-2 edge pass ----
              if max_phase >= 4:
                with (tc.tile_pool(name="wps2", bufs=6, space="PSUM") as win_ps,
                    tc.tile_pool(name="tps2", bufs=2, space="PSUM") as t_ps):
                  edge_layer(win_ps, t_ps, fs2_t, fd2_t, 1, h2T_own)

              # ---- phase 6: output projection ----
              if max_phase >= 6:
                with tc.tile_pool(name="ps3", bufs=2, space="PSUM") as psp:
                  for c0 in range(0, NB, 512):
                      cn = min(512, NB - c0)
                      ps = psp.tile([2, 512], F32, tag="out_psum", space="PSUM")
                      nc.tensor.matmul(out=ps[:, 0:cn], lhsT=wout_sb[:],
                                       rhs=h2T_own[:, c0:c0 + cn],
                                       start=True, stop=True)
                      ob = wpool.tile([2, 512], F32, tag="out_sb")
                      nc.vector.tensor_scalar_add(out=ob[:, 0:cn],
                                                  in0=ps[:, 0:cn],
                                                  scalar1=bout_sb[:, :])
                      nc.sync.dma_start(out=outT_d[:, c0:c0 + cn],
                                        in_=ob[:, 0:cn])

    nc.compile()
    return nc


def _prepare(src, dst):
    if "prog" not in _CACHE:
        sched, fs_idx, fd_idx, dstw = _prep_edges(src, dst)
        nc = _build_program(sched)
        _CACHE["prog"] = (nc, fs_idx, fd_idx, dstw)
    return _CACHE["prog"]


def make_in_maps(feature, src, dst, W_in, b_in, fc_src_W, fc_src_b,
                 fc_dst_W, fc_dst_b, attn, W_out, b_out):
    nc, fs_idx, fd_idx, dstw = _prepare(src, dst)
    feature = np.asarray(feature, np.float32)
    W_in = np.asarray(W_in, np.float32)
    b_in = np.asarray(b_in, np.float32)
    fc_src_W = np.asarray(fc_src_W, np.float32)
    fc_src_b = np.asarray(fc_src_b, np.float32)
    fc_dst_W = np.asarray(fc_dst_W, np.float32)
    fc_dst_b = np.asarray(fc_dst_b, np.float32)
    attn = np.asarray(attn, np.float32)
    W_out = np.asarray(W_out, np.float32)
    b_out = np.asarray(b_out, np.float32)

    wfs1 = (W_in @ fc_src_W[0]).astype(BF16)
    wfd1 = (W_in @ fc_dst_W[0]).astype(BF16)
    bfs1 = b_in @ fc_src_W[0] + fc_src_b[0]
    bfd1 = b_in @ fc_dst_W[0] + fc_dst_b[0]
    bias = np.stack([bfs1, bfd1, fc_src_b[1], fc_dst_b[1]])
    bias_rep = np.tile(bias[None], (P, 1, 1)).astype(BF16)
    arep = np.tile(attn.reshape(2, HF)[None], (P, 1, 1)).astype(BF16)
    iota_rep = np.tile(np.arange(P, dtype=np.float32), (P, MAXJ)).astype(BF16)
    ident = np.eye(P, dtype=np.float32).astype(BF16)
    featT = np.ascontiguousarray(feature.T).astype(BF16)

    common = {
        "featT": featT, "wfs1": wfs1, "wfd1": wfd1,
        "ws2": fc_src_W[1].astype(BF16), "wd2": fc_dst_W[1].astype(BF16),
        "bias": bias_rep, "arep": arep, "iota_rep": iota_rep, "ident": ident,
        "wout": W_out.astype(BF16),
        "bout": b_out.reshape(2, 1).astype(np.float32),
    }
    in_maps = []
    for c in range(N_CORES):
        m = dict(common)
        m["featT_own"] = np.ascontiguousarray(featT[:, c * NB:(c + 1) * NB])
        m["fs_idx"] = fs_idx[c]
        m["fd_idx"] = fd_idx[c]
        m["dstw"] = dstw[c].astype(BF16)
        in_maps.append(m)
    return nc, in_maps


def kernel(feature, src, dst, W_in, b_in, fc_src_W, fc_src_b,
           fc_dst_W, fc_dst_b, attn, W_out, b_out):
    from concourse import bass_utils

    nc, in_maps = make_in_maps(feature, src, dst, W_in, b_in, fc_src_W,
                               fc_src_b, fc_dst_W, fc_dst_b, attn, W_out,
                               b_out)
    res = bass_utils.run_bass_kernel_spmd(nc, in_maps,
                                          core_ids=list(range(N_CORES)))
    out = np.concatenate(
        [res.results[c]["outT"].T for c in range(N_CORES)], axis=0)
    return out.astype(np.float32)



# revision 20
# speedup vs baseline: 1.3809x; 1.0084x over previous
"""Trainium2 Bass kernel for 2-layer GATv2 (N=50000, E=800000, 128->64->64->2).

Strategy (edge-parallel, dst-sharded, 8 NeuronCores):
  * Host sorts edges by dst; core c owns dst nodes [c*N/8, (c+1)*N/8).
  * The softmax denominator factors out of the weighted sum, so each layer is
    ONE edge pass: gather fs[src], fd[dst]; score = a . lrelu(fs+fd);
    e = exp(score) (max-subtraction skipped -- scores are O(1)); a 0/1
    selection-matrix matmul scatter-adds [e*fs[src] | e] into per-128-node
    window PSUM accumulators; h = relu(u/s).
  * fs tables are per-core-replicated (src is global); fd tables are local.
  * dma_gather (Q7 SWDGE, int16 idx): fs indices split lo/hi at 32768; edges
    within each window group are reordered lo-first (sums are order-invariant).
  * Between layers: AllGather of h1^T pieces (ncfw collective).
"""
import sys
import numpy as np

sys.path.insert(0, "/opt/trn_rl_repo")

import ml_dtypes

BF16 = ml_dtypes.bfloat16

# ---------------- problem constants (hardcoded per contract) ----------------
N = 50000
E = 800000
IN_F = 128
HF = 64          # hidden feats
HEADS = 4
DH = 16
NEG_SLOPE = 0.2
N_CORES = 8
NB = N // N_CORES            # nodes per core
WIN = 128                    # window size (nodes)
WPC = (NB + WIN - 1) // WIN  # windows per core
GRP = 4                      # windows per psum group
ST = 16                      # tiles per DVE supertile
LO_SPLIT = 25000             # fs index split (balances q0/q1 descriptor load)
P = 128

_CACHE = {}
MAX_PHASE = 6
JUNK_SAFE = False
USE_PRELU = True  # cayman exp_and_others table holds Exp+Prelu+Copy+Relu together
EDGE_LEVEL = 3
REPEAT = 1
MAXJ = 32  # max scatter jobs per supertile chunk (batched sel build width)


def _wrap16(vals):
    """int array [n] (n % 16 == 0) -> [128, n/16] int16 wrapped+replicated."""
    b = vals.reshape(-1, 16).T.astype(np.int16)
    return np.tile(b, (8, 1))


def _prep_edges(src, dst):
    """Sort by dst, shard by dst range, group-level lo/hi packing.

    Stream per group of GRP windows: [lo(w0)..lo(w3) | hi(w0)..hi(w3)], padded
    to 128-edge tiles only at the lo/hi block level. A tile may span several
    windows; the per-(tile,window) scatter matmuls are emitted as "jobs" with
    their own dst-rel column (-1 outside the window).
    """
    src = np.asarray(src, dtype=np.int64)
    dst = np.asarray(dst, dtype=np.int64)
    perm = np.argsort(dst, kind="stable")
    se, de = src[perm], dst[perm]
    per_cw = [[None] * WPC for _ in range(N_CORES)]
    for c in range(N_CORES):
        a = np.searchsorted(de, c * NB, side="left")
        b = np.searchsorted(de, (c + 1) * NB, side="left")
        s_c, r_c = se[a:b], de[a:b] - c * NB
        w_c = r_c // WIN
        for w in range(WPC):
            m = w_c == w
            s_w, r_w = s_c[m], r_c[m]
            lo = s_w < LO_SPLIT
            per_cw[c][w] = (s_w[lo], r_w[lo], s_w[~lo], r_w[~lo])

    groups = []
    for g0 in range(0, WPC, GRP):
        ws = list(range(g0, min(g0 + GRP, WPC)))
        # per-core per-seg edge counts -> group tile counts (max over cores)
        lo_tot = [sum(len(per_cw[c][w][0]) for w in ws) for c in range(N_CORES)]
        hi_tot = [sum(len(per_cw[c][w][2]) for w in ws) for c in range(N_CORES)]
        T_lo = max(-(-n // P) for n in lo_tot)
        T_hi = max(-(-n // P) for n in hi_tot)
        gt = T_lo + T_hi
        # jobs: union over cores of (tile, w) touched
        jobs_set = {}
        for c in range(N_CORES):
            pos = 0
            for seg, base in ((0, 0), (2, T_lo * P)):
                pos = base
                for w in ws:
                    n = len(per_cw[c][w][seg])
                    if n:
                        for t in range(pos // P, -(-(pos + n) // P)):
                            jobs_set[(t, w)] = True
                    pos += n
        jobs = sorted(jobs_set.keys())
        first_j, last_j = {}, {}
        for j, (t, w) in enumerate(jobs):
            if w not in first_j:
                first_j[w] = j
            last_j[w] = j
        groups.append({"ws": ws, "gt": gt, "T_lo": T_lo, "T_hi": T_hi,
                       "jobs": jobs, "first_j": first_j, "last_j": last_j})
    TT = sum(g["gt"] for g in groups)
    NJ = sum(len(g["jobs"]) for g in groups)

    fs_idx = np.zeros((N_CORES, P, TT * 8), np.int16)
    fd_idx = np.zeros((N_CORES, P, TT * 8), np.int16)
    dstw = np.full((N_CORES, P, NJ), -1.0, np.float32)
    for c in range(N_CORES):
        col = 0
        j_base = 0
        for g in groups:
            gt, T_lo = g["gt"], g["T_lo"]
            s_all = np.zeros(gt * P, np.int64)
            d_all = np.zeros(gt * P, np.int64)
            wof = np.full(gt * P, -1, np.int64)   # window of each slot
            r_all = np.zeros(gt * P, np.int64)
            for seg, base in ((0, 0), (2, T_lo * P)):
                pos = base
                for w in g["ws"]:
                    s_w = per_cw[c][w][seg]
                    r_w = per_cw[c][w][seg + 1]
                    n = len(s_w)
                    s_all[pos:pos + n] = s_w - (LO_SPLIT if seg else 0)
                    d_all[pos:pos + n] = r_w
                    r_all[pos:pos + n] = r_w
                    wof[pos:pos + n] = w
                    pos += n
            fs_idx[c, :, col:col + T_lo * 8] = _wrap16(s_all[:T_lo * P])
            if gt - T_lo:
                fs_idx[c, :, col + T_lo * 8:col + gt * 8] = \
                    _wrap16(s_all[T_lo * P:])
            fd_idx[c, :, col:col + gt * 8] = _wrap16(d_all)
            col += gt * 8
            # dstw per job
            for j, (t, w) in enumerate(g["jobs"]):
                sl = slice(t * P, (t + 1) * P)
                v = np.where(wof[sl] == w, r_all[sl] - w * WIN, -1.0)
                dstw[c, :, j_base + j] = v
            j_base += len(g["jobs"])
    return {"groups": groups, "TT": TT, "NJ": NJ}, fs_idx, fd_idx, dstw


def _build_program(sched):
    import concourse.bacc as bacc
    import concourse.mybir as mybir
    import concourse.tile as tile

    BF = mybir.dt.bfloat16
    F32 = mybir.dt.float32
    I16 = mybir.dt.int16
    AF = mybir.ActivationFunctionType
    OP = mybir.AluOpType
    AX = mybir.AxisListType

    TT = sched["TT"]
    NJ = sched["NJ"]
    groups = sched["groups"]

    nc = bacc.Bacc("TRN2", target_bir_lowering=False, debug=False,
                   num_devices=N_CORES, num_swdge_queues=4)

    featT = nc.dram_tensor("featT", [IN_F, N], BF, kind="ExternalInput").ap()
    featT_own = nc.dram_tensor("featT_own", [IN_F, NB], BF,
                               kind="ExternalInput").ap()
    fs_idx_d = nc.dram_tensor("fs_idx", [P, TT * 8], I16,
                              kind="ExternalInput").ap()
    fd_idx_d = nc.dram_tensor("fd_idx", [P, TT * 8], I16,
                              kind="ExternalInput").ap()
    dstw_d = nc.dram_tensor("dstw", [P, NJ], BF, kind="ExternalInput").ap()
    wfs1_d = nc.dram_tensor("wfs1", [IN_F, HF], BF, kind="ExternalInput").ap()
    wfd1_d = nc.dram_tensor("wfd1", [IN_F, HF], BF, kind="ExternalInput").ap()
    ws2_d = nc.dram_tensor("ws2", [HF, HF], BF, kind="ExternalInput").ap()
    wd2_d = nc.dram_tensor("wd2", [HF, HF], BF, kind="ExternalInput").ap()
    bias_d = nc.dram_tensor("bias", [P, 4, HF], BF, kind="ExternalInput").ap()
    arep_d = nc.dram_tensor("arep", [P, 2, HF], BF, kind="ExternalInput").ap()
    iota_rep_d = nc.dram_tensor("iota_rep", [P, MAXJ * P], BF,
                                kind="ExternalInput").ap()
    ident_d = nc.dram_tensor("ident", [P, P], BF, kind="ExternalInput").ap()
    wout_d = nc.dram_tensor("wout", [HF, 2], BF, kind="ExternalInput").ap()
    bout_d = nc.dram_tensor("bout", [2, 1], F32, kind="ExternalInput").ap()
    outT_d = nc.dram_tensor("outT", [2, NB], F32, kind="ExternalOutput").ap()

    fs1_t = nc.dram_tensor("fs1_t", [N, P], BF).ap()   # cols 0:64 live
    fd1_t = nc.dram_tensor("fd1_t", [NB, P], BF).ap()
    fs2_own = nc.dram_tensor("fs2_own", [NB, P], BF).ap()
    fs2_t = nc.dram_tensor("fs2_t", [N, P], BF, addr_space="Shared").ap()
    fd2_t = nc.dram_tensor("fd2_t", [NB, P], BF).ap()

    with tile.TileContext(nc) as tc:
        with (
            tc.tile_pool(name="const", bufs=1) as cpool,
            tc.tile_pool(name="work", bufs=2) as wpool,
            tc.tile_pool(name="gath", bufs=3) as gpool,
        ):
            def cload(name, shape, dt_, src_ap):
                t = cpool.tile(shape, dt_, tag=name)
                nc.sync.dma_start(out=t[:], in_=src_ap)
                return t

            dstw_sb = cload("dstw_sb", [P, NJ], BF, dstw_d[:, :])
            wfs1_sb = cload("wfs1_sb", [IN_F, HF], BF, wfs1_d[:, :])
            wfd1_sb = cload("wfd1_sb", [IN_F, HF], BF, wfd1_d[:, :])
            ws2_sb = cload("ws2_sb", [HF, HF], BF, ws2_d[:, :])
            wd2_sb = cload("wd2_sb", [HF, HF], BF, wd2_d[:, :])
            bias_sb = cload("bias_sb", [P, 4, HF], BF, bias_d[:, :, :])
            arep_sb = cload("arep_sb", [P, 2, HF], BF, arep_d[:, :, :])
            iota_rep_sb = cload("iota_rep_sb", [P, MAXJ * P], BF,
                                iota_rep_d[:, :])
            ident_sb = cload("ident_sb", [P, P], BF, ident_d[:, :])
            wout_sb = cload("wout_sb", [HF, 2], BF, wout_d[:, :])
            bout_sb = cload("bout_sb", [2, 1], F32, bout_d[:, :])
            h1T_own = cpool.tile([HF, NB], BF, tag="h1T_own")
            h2T_own = cpool.tile([HF, NB], BF, tag="h2T_own")

            def project(psp, dst_table, n_rows, row0, lhsT_of, w_sb, bias_idx):
                """dst_table[row0+i, 0:64] = lhsT(i)^T @ w + bias (batches)."""
                BATCH = 8 * P
                for b0 in range(0, n_rows, BATCH):
                    bn = min(BATCH, n_rows - b0)
                    nch = -(-bn // P)
                    ps = psp.tile([P, 8 * HF], F32, tag="proj_psum",
                                  space="PSUM")
                    for k in range(nch):
                        c0 = b0 + k * P
                        cn = min(P, n_rows - c0)
                        nc.tensor.matmul(
                            out=ps[0:cn, k * HF:(k + 1) * HF],
                            lhsT=lhsT_of(c0, cn), rhs=w_sb[:],
                            start=True, stop=True)
                    ob = wpool.tile([P, 8, P], BF, tag="proj_out")
                    if JUNK_SAFE:
                        nc.vector.memset(ob[:, :, HF:P], 0.0)
                    wcols = P if JUNK_SAFE else HF
                    nc.vector.tensor_add(
                        out=ob[:, 0:nch, 0:HF],
                        in0=ps[:].rearrange("p (k f) -> p k f", k=8)[:, 0:nch, :],
                        in1=bias_sb[:, bias_idx, :].unsqueeze(1)
                            .to_broadcast([P, nch, HF]))
                    # table rows are 128B writes at 256B pitch; spread the
                    # packet load over two otherwise-idle HWDGE rings
                    out_eng = nc.scalar if (b0 // BATCH) % 2 else nc.sync
                    nf = bn // P
                    if nf:
                        out_eng.dma_start(
                            out=dst_table[row0 + b0:row0 + b0 + nf * P, 0:wcols]
                                .rearrange("(k p) f -> p k f", p=P),
                            in_=ob[:, 0:nf, 0:wcols])
                    if bn - nf * P:
                        out_eng.dma_start(
                            out=dst_table[row0 + b0 + nf * P:row0 + b0 + bn,
                                          0:wcols],
                            in_=ob[0:bn - nf * P, nf, 0:wcols])

            def edge_layer(win_ps, hT_ps_pool, fs_table, fd_table, a_idx,
                           hT_own):
                t_base = 0
                col = 0
                j_base = 0
                for g in groups:
                    gt = g["gt"]
                    n_lo = g["T_lo"]
                    fsg = gpool.tile([P, gt, P], BF, tag="fsg")
                    fdg = gpool.tile([P, gt, P], BF, tag="fdg")
                    fs_ix = gpool.tile([P, gt * 8], I16, tag="fs_ix")
                    nc.sync.dma_start(out=fs_ix[:],
                                      in_=fs_idx_d[:, col:col + gt * 8])
                    fd_ix = gpool.tile([P, gt * 8], I16, tag="fd_ix")
                    nc.sync.dma_start(out=fd_ix[:],
                                      in_=fd_idx_d[:, col:col + gt * 8])
                    gt2 = gt // 2
                    if n_lo and EDGE_LEVEL >= 0:
                        nc.gpsimd.dma_gather(
                            fsg[:, 0:n_lo, :], fs_table[0:LO_SPLIT, :],
                            fs_ix[:, 0:n_lo * 8], n_lo * P, n_lo * P, P,
                            single_packet=False, queue_num=0)
                    if gt - n_lo and EDGE_LEVEL >= 0:
                        nc.gpsimd.dma_gather(
                            fsg[:, n_lo:gt, :], fs_table[LO_SPLIT:N, :],
                            fs_ix[:, n_lo * 8:gt * 8],
                            (gt - n_lo) * P, (gt - n_lo) * P, P,
                            single_packet=False, queue_num=1)
                    if gt2 and EDGE_LEVEL >= 0:
                        nc.gpsimd.dma_gather(
                            fdg[:, 0:gt2, :], fd_table[:, :],
                            fd_ix[:, 0:gt2 * 8], gt2 * P, gt2 * P, P,
                            single_packet=False, queue_num=2)
                    if gt - gt2 and EDGE_LEVEL >= 0:
                        nc.gpsimd.dma_gather(
                            fdg[:, gt2:gt, :], fd_table[:, :],
                            fd_ix[:, gt2 * 8:gt * 8],
                            (gt - gt2) * P, (gt - gt2) * P, P,
                            single_packet=False, queue_num=3)
                    col += gt * 8

                    jobs = g["jobs"]
                    first_j, last_j = g["first_j"], g["last_j"]
                    psums = {w: win_ps.tile([P, HF + HEADS], F32, name="win_psum",
                                            tag="win_psum", space="PSUM")
                             for w in first_j}

                    for s0 in range(0, gt, ST):
                        if EDGE_LEVEL < 1:
                            break
                        sn = min(ST, gt - s0)
                        fs_v = fsg[:, s0:s0 + sn, 0:HF]
                        fd_v = fdg[:, s0:s0 + sn, 0:HF]
                        t0 = wpool.tile([P, ST, HF], BF, tag="t0")
                        nc.vector.tensor_add(out=t0[:, 0:sn, :], in0=fs_v,
                                             in1=fd_v)
                        t1 = wpool.tile([P, ST, HF], BF, tag="t1")
                        if USE_PRELU:
                            nc.scalar.activation(
                                out=t1[:, 0:sn, :], in_=t0[:, 0:sn, :],
                                func=AF.Prelu, alpha=NEG_SLOPE)
                        else:
                            nc.vector.scalar_tensor_tensor(
                                out=t1[:, 0:sn, :], in0=t0[:, 0:sn, :],
                                scalar=NEG_SLOPE, in1=t0[:, 0:sn, :],
                                op0=OP.mult, op1=OP.max)
                        t2 = wpool.tile([P, ST, HF], BF, tag="t2")
                        nc.vector.tensor_mul(
                            out=t2[:, 0:sn, :], in0=t1[:, 0:sn, :],
                            in1=arep_sb[:, a_idx, :].unsqueeze(1)
                                .to_broadcast([P, sn, HF]))
                        t3 = wpool.tile([P, ST, HEADS, DH // 2], BF,
                                        tag="t3")
                        t2v = t2[:, 0:sn, :].rearrange(
                            "p t (h d) -> p (t h) d", d=DH)
                        nc.vector.tensor_add(
                            out=t3[:, 0:sn, :, :]
                                .rearrange("p t h d -> p (t h) d"),
                            in0=t2v[:, :, 0:DH // 2],
                            in1=t2v[:, :, DH // 2:DH])
                        sc = wpool.tile([P, ST * HEADS], F32, tag="sc")
                        nc.vector.tensor_reduce(
                            out=sc[:, 0:sn * HEADS]
                                .rearrange("p (t h) -> p t h", h=HEADS),
                            in_=t3[:, 0:sn, :, :]
                                .rearrange("p t h d -> p (t h) d"),
                            op=OP.add, axis=AX.X)
                        rhs = wpool.tile([P, ST, HF + HEADS], BF, tag="rhs")
                        nc.scalar.activation(
                            out=rhs[:, 0:sn, HF:HF + HEADS],
                            in_=sc[:, 0:sn * HEADS]
                                .rearrange("p (t h) -> p t h", h=HEADS),
                            func=AF.Exp)
                        nc.vector.tensor_mul(
                            out=rhs[:, 0:sn, 0:HF]
                                .rearrange("p t (h d) -> p t h d", d=DH),
                            in0=fs_v.rearrange("p t (h d) -> p t h d", d=DH),
                            in1=rhs[:, 0:sn, HF:HF + HEADS].unsqueeze(3)
                                .to_broadcast([P, sn, HEADS, DH]))
                        chunk_jobs = [(j, t, w) for j, (t, w) in
                                      enumerate(jobs) if s0 <= t < s0 + sn]
                        nJ = len(chunk_jobs)
                        assert nJ <= MAXJ, (nJ, MAXJ)
                        sel = wpool.tile([P, MAXJ, P], BF, tag="sel")
                        if EDGE_LEVEL >= 2 and nJ:
                            j0c = chunk_jobs[0][0]
                            nc.vector.tensor_tensor(
                                out=sel[:, 0:nJ, :],
                                in0=iota_rep_sb[:, 0:nJ * P]
                                    .rearrange("p (j n) -> p j n", n=P),
                                in1=dstw_sb[:, j_base + j0c:
                                            j_base + j0c + nJ]
                                    .unsqueeze(2).to_broadcast([P, nJ, P]),
                                op=OP.is_equal)
                            for js, (j, t, w) in enumerate(chunk_jobs):
                                nc.tensor.matmul(
                                    out=psums[w][:], lhsT=sel[:, js, :],
                                    rhs=rhs[:, t - s0, :],
                                    start=(j == first_j[w]),
                                    stop=(j == last_j[w]))

                    for w in g["ws"]:
                        if w not in first_j or EDGE_LEVEL < 2:
                            continue
                        ps = psums[w]
                        nw = min(WIN, NB - w * WIN)
                        s_eps = wpool.tile([P, HEADS], F32, tag="s_eps")
                        nc.vector.tensor_scalar_add(
                            out=s_eps[:], in0=ps[:, HF:HF + HEADS],
                            scalar1=1e-20)
                        s_inv = wpool.tile([P, HEADS], F32, tag="s_inv")
                        nc.vector.reciprocal(out=s_inv[:], in_=s_eps[:])
                        hw_ = wpool.tile([P, HF], BF, tag="hw_")
                        nc.vector.tensor_mul(
                            out=hw_[:].rearrange("p (h d) -> p h d", d=DH),
                            in0=ps[:, 0:HF].rearrange("p (h d) -> p h d",
                                                      d=DH),
                            in1=s_inv[:].unsqueeze(2)
                                .to_broadcast([P, HEADS, DH]))
                        hrel = wpool.tile([P, HF], BF, tag="hrel")
                        nc.scalar.activation(out=hrel[:], in_=hw_[:],
                                             func=AF.Relu)
                        if EDGE_LEVEL < 3:
                            continue
                        hT_ps = hT_ps_pool.tile([HF, P], BF, tag="hT_ps",
                                                space="PSUM")
                        nc.tensor.transpose(out=hT_ps[:], in_=hrel[:],
                                            identity=ident_sb[:])
                        nc.vector.tensor_copy(
                            out=hT_own[:, w * WIN:w * WIN + nw],
                            in_=hT_ps[:, 0:nw])
                    t_base += gt
                    j_base += len(jobs)

            def batched_lhsT(src_ap, width, tag):
                cache = {}

                def f(c0, cn):
                    b0 = (c0 // (8 * P)) * (8 * P)
                    if cache.get("b0") != b0:
                        bw = min(8 * P, width - b0)
                        t = wpool.tile([src_ap.shape[0], 8 * P], BF, tag=tag)
                        nc.sync.dma_start(out=t[:, 0:bw],
                                          in_=src_ap[:, b0:b0 + bw])
                        cache["b0"], cache["t"] = b0, t
                    return cache["t"][:, c0 - b0:c0 - b0 + cn]
                return f

            # ---- phase 1: layer-1 projections ----
            max_phase = MAX_PHASE
            for _rep in range(REPEAT):
              with tc.tile_pool(name="ps1", bufs=2, space="PSUM") as psp:
                  project(psp, fs1_t, N, 0,
                          batched_lhsT(featT, N, "featT_chunk"), wfs1_sb, 0)
                  project(psp, fd1_t, NB, 0,
                          batched_lhsT(featT_own, NB, "featT_own_chunk"),
                          wfd1_sb, 1)

              # ---- phase 2: layer-1 edge pass ----
              if max_phase >= 2:
                with (tc.tile_pool(name="wps1", bufs=6, space="PSUM") as win_ps,
                    tc.tile_pool(name="tps1", bufs=2, space="PSUM") as t_ps):
                  edge_layer(win_ps, t_ps, fs1_t, fd1_t, 0, h1T_own)

              # ---- phase 3+4: layer-2 projections (own rows) + AllGather ----
              if max_phase >= 3:
                with tc.tile_pool(name="ps2", bufs=2, space="PSUM") as psp:
                  project(psp, fd2_t, NB, 0,
                          lambda c0, cn: h1T_own[:, c0:c0 + cn], wd2_sb, 3)
                  project(psp, fs2_own, NB, 0,
                          lambda c0, cn: h1T_own[:, c0:c0 + cn], ws2_sb, 2)
                nc.gpsimd.collective_compute(
                  "AllGather", OP.bypass, ins=[fs2_own[:, :]],
                  outs=[fs2_t[:, :]],
                  replica_groups=[list(range(N_CORES))])

              # ---- phase 5: layer-2 edge pass ----
              if max_phase >= 4:
                with (tc.tile_pool(name="wps2", bufs=6, space="PSUM") as win_ps,
                    tc.tile_pool(name="tps2", bufs=2, space="PSUM") as t_ps):
                  edge_layer(win_ps, t_ps, fs2_t, fd2_t, 1, h2T_own)

              # ---- phase 6: output projection ----
              if max_phase >= 6:
                with tc.tile_pool(name="ps3", bufs=2, space="PSUM") as psp:
                  for c0 in range(0, NB, 512):
                      cn = min(512, NB - c0)
                      ps = psp.tile([2, 512], F32, tag="out_psum", space="PSUM")
                      nc.tensor.matmul(out=ps[:, 0:cn], lhsT=wout_sb[:],
                                       rhs=h2T_own[:, c0:c0 + cn],
                                       start=True, stop=True)
                      ob = wpool.tile([2, 512], F32, tag="out_sb")
                      nc.vector.tensor_scalar_add(out=ob[:, 0:cn],
                                                  in0=ps[:, 0:cn],
                                                  scalar1=bout_sb[:, :])
                      nc.sync.dma_start(out=outT_d[:, c0:c0 + cn],
                                        in_=ob[:, 0:cn])

    nc.compile()
    return nc


def _prepare(src, dst):
    if "prog" not in _CACHE:
        sched, fs_idx, fd_idx, dstw = _prep_edges(src, dst)
        nc = _build_program(sched)
        _CACHE["prog"] = (nc, fs_idx, fd_idx, dstw)
    return _CACHE["prog"]


def make_in_maps(feature, src, dst, W_in, b_in, fc_src_W, fc_src_b,
                 fc_dst_W, fc_dst_b, attn, W_out, b_out):
    nc, fs_idx, fd_idx, dstw = _prepare(src, dst)
    feature = np.asarray(feature, np.float32)
    W_in = np.asarray(W_in, np.float32)
    b_in = np.asarray(b_in, np.float32)
    fc_src_W = np.asarray(fc_src_W, np.float32)
    fc_src_b = np.asarray(fc_src_b, np.float32)
    fc_dst_W = np.asarray(fc_dst_W, np.float32)
    fc_dst_b = np.asarray(fc_dst_b, np.float32)
    attn = np.asarray(attn, np.float32)
    W_out = np.asarray(W_out, np.float32)
    b_out = np.asarray(b_out, np.float32)

    wfs1 = (W_in @ fc_src_W[0]).astype(BF16)
    wfd1 = (W_in @ fc_dst_W[0]).astype(BF16)
    bfs1 = b_in @ fc_src_W[0] + fc_src_b[0]
    bfd1 = b_in @ fc_dst_W[0] + fc_dst_b[0]
    bias = np.stack([bfs1, bfd1, fc_src_b[1], fc_dst_b[1]])
    bias_rep = np.tile(bias[None], (P, 1, 1)).astype(BF16)
    arep = np.tile(attn.reshape(2, HF)[None], (P, 1, 1)).astype(BF16)
    iota_rep = np.tile(np.arange(P, dtype=np.float32), (P, MAXJ)).astype(BF16)
    ident = np.eye(P, dtype=np.float32).astype(BF16)
    featT = np.ascontiguousarray(feature.T).astype(BF16)

    common = {
        "featT": featT, "wfs1": wfs1, "wfd1": wfd1,
        "ws2": fc_src_W[1].astype(BF16), "wd2": fc_dst_W[1].astype(BF16),
        "bias": bias_rep, "arep": arep, "iota_rep": iota_rep, "ident": ident,
        "wout": W_out.astype(BF16),
        "bout": b_out.reshape(2, 1).astype(np.float32),
    }
    in_maps = []
    for c in range(N_CORES):
        m = dict(common)
        m["featT_own"] = np.ascontiguousarray(featT[:, c * NB:(c + 1) * NB])
        m["fs_idx"] = fs_idx[c]
        m["fd_idx"] = fd_idx[c]
        m["dstw"] = dstw[c].astype(BF16)
        in_maps.append(m)
    return nc, in_maps


def kernel(feature, src, dst, W_in, b_in, fc_src_W, fc_src_b,
           fc_dst_W, fc_dst_b, attn, W_out, b_out):
    from concourse import bass_utils

    nc, in_maps = make_in_maps(feature, src, dst, W_in, b_in, fc_src_W,
                               fc_src_b, fc_dst_W, fc_dst_b, attn, W_out,
                               b_out)
    res = bass_utils.run_bass_kernel_spmd(nc, in_maps,
                                          core_ids=list(range(N_CORES)))
    out = np.concatenate(
        [res.results[c]["outT"].T for c in range(N_CORES)], axis=0)
    return out.astype(np.float32)

